# revision 2
# baseline (speedup 1.0000x reference)
"""Trainium2 Bass kernel v2 for nn_BlockRC3 (PRM dilated-conv stem + Token_performer).

Contract: kernel(**inputs) takes FULL unsharded inputs (x [4,65536,64] fp32 + weights),
returns FULL output [4,16384,320] fp32. Data-parallel over 8 NeuronCores, each core
half an image (8192 tokens); one pairwise AllReduce of performer sums (kptv+ks).

v2 changes vs baseline (same math, ~2.5x lower modeled device time):
  - single activation-table set per phase: LN rstd via Ln+Exp (natural_log_exp set)
    instead of Sqrt (+reciprocal), so phase B/C1 never reload ACT tables.
  - v/qp/ya2/h2 stay in SBUF; qp transposed on the PE (128x128 via identity
    matmul) instead of DRAM DMA round trip + DmaTranspose.
  - phase C restructured token-major: per-token scalars (1/D, LN2 stats) are
    per-partition ops, no DMA broadcast round trips; proj bias via vpj,
    mlp2 bias via ones-row in lhsT; output written token-major [8192,320].
  - kptv+ks fused into one PSUM accumulation (ones column in v_sb).
  - elementwise work split across DVE / ACT / Pool(gpsimd, SBUF-only ops).
"""

import math
import os

import numpy as np
import ml_dtypes

import concourse.bacc as bacc
import concourse.mybir as mybir
import concourse.tile as tile
from concourse.bass_utils import run_bass_kernel_spmd

FP32 = mybir.dt.float32
BF16 = mybir.dt.bfloat16
AF = mybir.ActivationFunctionType
ALU = mybir.AluOpType

B, N_IN, CIN = 4, 65536, 64
H = W = 256
EMB, ED, M = 320, 160, 160
T_CORE = 8192            # tokens per core (half image)
NTILE = 64               # 128-token tiles per core
NBLK = 16                # 512-token blocks per core
PR, PC = 66, 130         # parity plane rows/cols (from padded 131x260 input)
PLANE = PR * PC
EPS_LN = 1e-5
EPS_ATTN = 1e-8
LNM_HALF = 0.5 * math.log(M)

WBIG_COLS = 1282
# wbig column layout (3 psum banks):
# b0 <- cols 0:480    : v 0:320 | wtxk 320:480
# b1 <- cols 480:960  : wtxq 0:160 | Zk 160:480
# b2 <- cols 960:1282 : Zq 0:320 | mu 320 | Et2 321 (zero col, t^2 matmul accum)

PERM = np.concatenate([
    np.arange(0, 128),          # conv1 o 0:128
    np.arange(160, 288),        # conv2 o 0:128
    np.arange(128, 160),        # conv1 o 128:160
    np.arange(288, 320),        # conv2 o 128:160
])

_BF = ml_dtypes.bfloat16


def _bf16(a):
    return np.ascontiguousarray(a, dtype=np.float32).astype(_BF)


def _shift_flat(plane, delta):
    out = np.zeros_like(plane)
    out[:, : PLANE - delta] = plane[:, delta:]
    return out


def host_prepare_weights(inp):
    g1 = inp["ln1_g"].astype(np.float64)
    b1 = inp["ln1_b"].astype(np.float64)
    kqv_w = inp["kqv_w"].astype(np.float64)          # [960, 320]
    kqv_b = inp["kqv_b"].astype(np.float64)
    pw = inp["perf_w"].astype(np.float64)            # [160, 320]

    Wp = kqv_w * g1[None, :]
    b_fold = kqv_b + kqv_w @ b1
    s = Wp.sum(axis=1)
    Wpp = Wp - s[:, None] / EMB
    Wk, Wq, Wv = Wpp[0:EMB], Wpp[EMB:2 * EMB], Wpp[2 * EMB:]
    PK = pw @ Wk
    PQ = pw @ Wq
    bias_zero = (np.abs(b_fold).max() == 0.0)

    p = PERM
    Wk_s, Wq_s, Wv_s = Wk[:, p], Wq[:, p], Wv[:, p]
    PK_s, PQ_s = PK[:, p], PQ[:, p]

    wbig = np.zeros((EMB, WBIG_COLS), np.float64)
    wbig[:, 0:320] = Wv_s.T
    wbig[:, 320:480] = PK_s.T
    wbig[:, 480:640] = PQ_s.T
    wbig[:, 640:960] = Wk_s.T
    wbig[:, 960:1280] = Wq_s.T
    wbig[:, 1280] = 1.0   # mu column (device scales by 1/EMB)
    # col 1281 zero: Et2 accumulated by the t^2 matmuls

    # conv weights: tap-pair stationary tiles (same as baseline)
    w1 = inp["conv_w1"].astype(np.float64)
    w2 = inp["conv_w2"].astype(np.float64)

    def tapw(w, dy, dx, osl):
        return np.ascontiguousarray(w[osl, :, dy, dx].T)

    def pairw(w, tapA, tapB, osl):
        return np.concatenate([tapw(w, *tapA, osl), tapw(w, *tapB, osl)], axis=0)

    lo, hi = slice(0, 128), slice(128, 160)
    conv = {}
    c1_pairs = [("S3", 0, (0, 0), (0, 2)), ("S3", 130, (2, 0), (2, 2)),
                ("S4", 1, (0, 1), (2, 1)), ("S5", 130, (1, 0), (1, 2))]
    c1_single = ("S1u", 131, (1, 1))
    c2_pairs = [("S1", 0, (0, 0), (0, 1)), ("S1", 130, (1, 0), (1, 1)),
                ("S1", 260, (2, 0), (2, 1)), ("S2", 2, (0, 2), (1, 2))]
    c2_single = ("S1l", 261, (2, 2))
    for osl, tag in ((lo, "lo"), (hi, "hi")):
        conv[f"c1_{tag}_pairs"] = [(st, off, _bf16(pairw(w1, tA, tB, osl)))
                                   for st, off, tA, tB in c1_pairs]
        conv[f"c1_{tag}_single"] = (c1_single[0], c1_single[1],
                                    _bf16(tapw(w1, *c1_single[2], osl)))
        conv[f"c2_{tag}_pairs"] = [(st, off, _bf16(pairw(w2, tA, tB, osl)))
                                   for st, off, tA, tB in c2_pairs]
        conv[f"c2_{tag}_single"] = (c2_single[0], c2_single[1],
                                    _bf16(tapw(w2, *c2_single[2], osl)))

    cb = np.concatenate([inp["conv_b1"], inp["conv_b2"]]).astype(np.float64)[PERM]

    # LN2 + MLP folds (token-major LN2: only the gain folds into w1)
    g2 = inp["ln2_g"].astype(np.float64)
    b2 = inp["ln2_b"].astype(np.float64)
    w_1 = inp["mlp_w1"].astype(np.float64)           # [320h, 320e]
    b_1 = inp["mlp_b1"].astype(np.float64)
    w_2 = inp["mlp_w2"].astype(np.float64)           # [320o, 320h]
    b_2 = inp["mlp_b2"].astype(np.float64)
    V1g = w_1 * g2[None, :]                          # [320h, 320e]
    b1f = b_1 + w_1 @ b2
    proj_w = inp["proj_w"].astype(np.float64)
    proj_b = inp["proj_b"].astype(np.float64)

    w2t_ext = np.zeros((321, EMB), np.float64)       # [h(+1), o]
    w2t_ext[0:320] = w_2.T
    w2t_ext[320] = b_2

    out = dict(
        wbig=_bf16(wbig),
        conv=conv,
        conv_bias=cb.astype(np.float32),
        bias_zero=bias_zero,
        v1gT=_bf16(V1g.T),                            # [320e, 320h]
        b1f=b1f.astype(np.float32),
        w2t=_bf16(w2t_ext),                           # [321h, 320o]
        projwT=_bf16(proj_w.T),                       # [320e, 320o]
        proj_b=proj_b.astype(np.float32),
        pjb_zero=(np.abs(proj_b).max() == 0.0),
        b2_zero=(np.abs(b_2).max() == 0.0),
    )
    return out


def host_prepare_core_input(x, core):
    b, half = core // 2, core % 2
    xi = np.ascontiguousarray(x[b].reshape(H, W, CIN).transpose(2, 0, 1))
    r0 = 128 * half - 2
    pad = np.zeros((CIN, 131, 260), np.float32)
    rlo, rhi = max(r0, 0), min(r0 + 131, H)
    pad[:, rlo - r0:rhi - r0, 2:258] = xi[:, rlo:rhi, :]
    ee = pad[:, 0::2, 0::2]
    eo = pad[:, 0::2, 1::2]
    oe = np.zeros((CIN, PR, PC), np.float32); oe[:, :65] = pad[:, 1::2, 0::2]
    oo = np.zeros((CIN, PR, PC), np.float32); oo[:, :65] = pad[:, 1::2, 1::2]
    ee = ee.reshape(CIN, PLANE); eo = eo.reshape(CIN, PLANE)
    oe = oe.reshape(CIN, PLANE); oo = oo.reshape(CIN, PLANE)
    stacks = {
        "S1": np.concatenate([ee, _shift_flat(ee, 1)], axis=0),
        "S2": np.concatenate([ee, _shift_flat(ee, 130)], axis=0),
        "S3": np.concatenate([oo, _shift_flat(oo, 1)], axis=0),
        "S4": np.concatenate([oe, _shift_flat(oe, 130)], axis=0),
        "S5": np.concatenate([eo, _shift_flat(eo, 1)], axis=0),
    }
    return {k: _bf16(v) for k, v in stacks.items()}


# ---------------------------------------------------------------------------
# device kernel builder
# ---------------------------------------------------------------------------

def build_nc(pjb_zero=True, b2_zero=True):
    import contextlib
    import concourse.bass as bass

    nc = bacc.Bacc(None, target_bir_lowering=False)

    # Restrict the activation-table chooser to the two sets that each cover a
    # whole phase (indices preserved; other sets emptied so the fixpoint pass
    # can't alternate between per-function tables, which would reload the
    # 1.3us ACT table per tile).
    KEEP = {"natural_log_exp_and_others", "gelu_and_others"}
    from concourse.hw_specs import get_activation_tables
    import bass_rust as _bass_rust_mod

    def _patched_insert_act_table_loads():
        has_activation = any(
            isinstance(i, mybir.InstActivation)
            for b_ in nc.main_func.blocks
            for i in b_.instructions
        )
        if not has_activation:
            return
        tables = [(name, (funcs if name in KEEP else set()))
                  for name, funcs in get_activation_tables(nc.m.arch).items()]
        _bass_rust_mod.insert_act_table_loads(nc, tables)

    nc.insert_act_table_loads = _patched_insert_act_table_loads

    def din(name, shape, dt=BF16):
        return nc.declare_dram_parameter(name, list(shape), dt, isOutput=False)

    stacks_ext = {s: din(f"stk_{s}", [128, PLANE]) for s in ("S1", "S2", "S3", "S4", "S5")}
    wbig_ext = [din("wbig0", [128, WBIG_COLS]), din("wbig1", [128, WBIG_COLS]),
                din("wbig2", [64, WBIG_COLS])]
    cw_ext = {}
    for cv in ("c1", "c2"):
        cw_ext[f"{cv}_lo_pairs"] = din(f"{cv}_lo_pairs", [128, 4 * 128])
        cw_ext[f"{cv}_hi_pairs"] = din(f"{cv}_hi_pairs", [128, 4 * 32])
    cw_ext["singles_lo"] = din("singles_lo", [128, 128])
    cw_ext["singles_hi"] = din("singles_hi", [128, 32])
    convb_ext = din("convb", [320], FP32)
    v1g_ext = din("v1gT", [320, 320])
    w2t_ext = din("w2t", [321, 320])
    pjt_ext = din("projwT", [320, 320])
    b1f_ext = din("b1f", [320], FP32)
    pjb_ext = None if pjb_zero else din("projb", [320], FP32)
    ident_ext = din("ident", [128, 128])

    out_ext = nc.declare_dram_parameter("out", [T_CORE, 320], FP32, isOutput=True)

    ECH = [(0, 128), (128, 128), (256, 64)]
    HCH = [(0, 128), (128, 128), (256, 64)]          # mlp hidden chunks

    with tile.TileContext(nc) as tc:
        with contextlib.ExitStack() as ctx:
            persist = ctx.enter_context(tc.tile_pool(name="persist", bufs=1))
            dram = ctx.enter_context(tc.tile_pool(name="dram", bufs=1, space="DRAM"))

            # ---- persistent constants ----
            eps_ln_t = persist.tile([128, 1], FP32)
            nc.vector.memset(eps_ln_t, EPS_LN)
            ones_t2 = [persist.tile([p, 1], FP32, name=f"ones_t2_{i}")
                       for i, (_, p) in enumerate(ECH)]
            for t in ones_t2:
                nc.vector.memset(t, 1.0 / EMB)
            convb_sb = persist.tile([128, 3], FP32)
            nc.sync.dma_start(out=convb_sb[:, 0:1], in_=convb_ext[0:128].rearrange("(b one) -> b one", one=1))
            nc.sync.dma_start(out=convb_sb[:, 1:2], in_=convb_ext[128:256].rearrange("(b one) -> b one", one=1))
            nc.sync.dma_start(out=convb_sb[0:64, 2:3], in_=convb_ext[256:320].rearrange("(b one) -> b one", one=1))
            b1f_sb = persist.tile([128, 3], FP32)
            for i, (o, p) in enumerate(HCH):
                nc.sync.dma_start(out=b1f_sb[0:p, i:i + 1], in_=b1f_ext[o:o + p].rearrange("(b one) -> b one", one=1))
            ident_sb = persist.tile([128, 128], BF16)
            nc.sync.dma_start(out=ident_sb, in_=ident_ext[:, :])
            if not pjb_zero:
                pjb_bc = persist.tile([128, 320], FP32)
                prow = pjb_ext.rearrange("(one c) -> one c", one=1)[0:1, :]
                nc.sync.dma_start(
                    out=pjb_bc,
                    in_=bass.AP(tensor=prow.tensor, offset=prow.offset,
                                ap=[[0, 128], [1, 320]]))

            cc_in = dram.tile([128, 480], FP32)
            cc_out = dram.tile([128, 480], FP32)

            # ========== v/qpT/ya2/h2 scope (phases B+C) ==========
            pbc = ctx.enter_context(tc.tile_pool(name="pbc", bufs=1))
            v_sb = pbc.tile([128, NTILE, 321], BF16, name="v_sb")
            nc.vector.memset(v_sb[:, :, 320:321], 1.0)   # ks ones column
            qpT0 = pbc.tile([128, T_CORE], BF16, name="qpT0")
            qpT1 = pbc.tile([32, T_CORE], BF16, name="qpT1")

            # ========== tc_sb scope (phases A+B) ==========
            ab_stack = ctx.enter_context(contextlib.ExitStack())
            pab = ab_stack.enter_context(tc.tile_pool(name="pab", bufs=1))
            tc_sb = [pab.tile([p, T_CORE], BF16, name=f"tc_sb_{i}")
                     for i, (_, p) in enumerate(ECH)]

            # =================== PHASE A: conv ===================
            with tc.tile_pool(name="convp", bufs=1) as convp, \
                 tc.tile_pool(name="cpsum", bufs=2, space="PSUM") as cpsum:
                cw = {}
                for k, ext in cw_ext.items():
                    t = convp.tile([128, ext.shape[1]], BF16, name=f"cw_{k}_sb")
                    nc.sync.dma_start(out=t, in_=ext[:, :])
                    cw[k] = t
                stk = {}
                for s in stacks_ext:
                    stk[s] = convp.tile([128, PLANE], BF16, name=f"stk_{s}_sb")
                # chunked + interleaved so conv blk0's rows (every stack's head)
                # land before the tails
                qtr = PLANE // 4
                for q in range(4):
                    lo = q * qtr
                    for s, ext in stacks_ext.items():
                        hi = PLANE if q == 3 else (q + 1) * qtr
                        nc.sync.dma_start(out=stk[s][:, lo:hi], in_=ext[:, lo:hi])

                def stack_view(name):
                    base = stk[name[:2]]
                    r = base.rearrange("p (r c) -> p r c", c=PC)
                    if name.endswith("u"):
                        return r[0:64]
                    if name.endswith("l"):
                        return r[64:128]
                    return r

                def conv_rhs(stname, flat_off, blk):
                    ro, co = divmod(flat_off, PC)
                    v = stack_view(stname)
                    h0 = blk * 4
                    return v[:, h0 + ro:h0 + ro + 4, co:co + 128]

                PAIR_DEFS = {
                    "c1": ([("S3", 0), ("S3", 130), ("S4", 1), ("S5", 130)], ("S1u", 131)),
                    "c2": ([("S1", 0), ("S1", 130), ("S1", 260), ("S2", 2)], ("S1l", 261)),
                }

                for blk in range(NBLK):
                    ps_lo1 = cpsum.tile([128, 512], FP32, tag="pslo1")
                    ps_lo2 = cpsum.tile([128, 512], FP32, tag="pslo2")
                    ps_hi = cpsum.tile([64, 512], FP32, tag="pshi")
                    for cvi, cv in enumerate(("c1", "c2")):
                        pairs, single = PAIR_DEFS[cv]
                        ps = (ps_lo1, ps_lo2)[cvi]
                        wlo = cw[f"{cv}_lo_pairs"]
                        whi = cw[f"{cv}_hi_pairs"]
                        for k, (st, off) in enumerate(pairs):
                            rhs = conv_rhs(st, off, blk)
                            nc.tensor.matmul(ps, wlo[:, k * 128:(k + 1) * 128], rhs,
                                             start=(k == 0), stop=False)
                            nc.tensor.matmul(ps_hi[cvi * 32:(cvi + 1) * 32, :],
                                             whi[:, k * 32:(k + 1) * 32], rhs,
                                             start=(k == 0), stop=False,
                                             tile_position=(0, 32 * cvi))
                        st, off = single
                        rhs = conv_rhs(st, off, blk)
                        wsl = cw["singles_lo"][cvi * 64:(cvi + 1) * 64, :]
                        wsh = cw["singles_hi"][cvi * 64:(cvi + 1) * 64, :]
                        nc.tensor.matmul(ps, wsl, rhs, start=False, stop=True,
                                         tile_position=(64 * cvi, 0))
                        nc.tensor.matmul(ps_hi[cvi * 32:(cvi + 1) * 32, :], wsh, rhs,
                                         start=False, stop=True,
                                         tile_position=(64 * cvi, 32 * cvi))
                    csl = slice(blk * 512, (blk + 1) * 512)
                    nc.scalar.activation(out=tc_sb[0][:, csl], in_=ps_lo1,
                                         func=AF.Gelu, bias=convb_sb[:, 0:1])
                    nc.scalar.activation(out=tc_sb[1][:, csl], in_=ps_lo2,
                                         func=AF.Gelu, bias=convb_sb[:, 1:2])
                    nc.scalar.activation(out=tc_sb[2][:, csl], in_=ps_hi,
                                         func=AF.Gelu, bias=convb_sb[0:64, 2:3])

            # =================== PHASE B: stage-1 + kptv + qp transpose ========
            with tc.tile_pool(name="pb", bufs=1) as pb, \
                 tc.tile_pool(name="spsum", bufs=6, space="PSUM") as spsum, \
                 tc.tile_pool(name="kpsum", bufs=1, space="PSUM") as kpsum, \
                 tc.tile_pool(name="tpsum", bufs=1, space="PSUM") as tpsum, \
                 tc.tile_pool(name="bwork", bufs=4) as bwork:
                wbig_sb = []
                for i, ext in enumerate(wbig_ext):
                    t = pb.tile([ext.shape[0], WBIG_COLS], BF16, name=f"wbig_sb_{i}")
                    nc.sync.dma_start(out=t, in_=ext[:, :])
                    wbig_sb.append(t)

                psA = kpsum.tile([128, 480], FP32)

                for i in range(NTILE):
                    tsl = slice(i * 128, (i + 1) * 128)
                    b0 = spsum.tile([128, 480], FP32, tag="s1", name="b0")
                    b1 = spsum.tile([128, 480], FP32, tag="s1", name="b1")
                    b2 = spsum.tile([128, 480], FP32, tag="s1", name="b2")
                    for kc in range(3):
                        lhsT = tc_sb[kc][:, tsl]
                        nc.tensor.matmul(b0, lhsT, wbig_sb[kc][:, 0:480],
                                         start=(kc == 0), stop=(kc == 2))
                        nc.tensor.matmul(b1, lhsT, wbig_sb[kc][:, 480:960],
                                         start=(kc == 0), stop=(kc == 2))
                        nc.tensor.matmul(b2[:, 0:322], lhsT, wbig_sb[kc][:, 960:1282],
                                         start=(kc == 0), stop=False)
                    for kc in range(3):
                        _, pch = ECH[kc]
                        t2 = bwork.tile([128, 128], FP32, tag="t2", name="t2")
                        nc.gpsimd.tensor_tensor(out=t2[0:pch, :], in0=tc_sb[kc][:, tsl],
                                                in1=tc_sb[kc][:, tsl], op=ALU.mult)
                        nc.tensor.matmul(b2[:, 321:322], t2[0:pch, :], ones_t2[kc],
                                         start=False, stop=(kc == 2))

                    # LN1 stats: var = Et2 - mu^2 ; rstd = exp(-0.5 ln(var+eps))
                    mu_s = bwork.tile([128, 1], FP32, tag="sc", bufs=16, name="mu_s")
                    nc.vector.tensor_scalar(out=mu_s, in0=b2[:, 320:321],
                                            scalar1=1.0 / EMB, scalar2=None, op0=ALU.mult)
                    musq = bwork.tile([128, 1], FP32, tag="sc", bufs=16, name="musq")
                    nc.vector.tensor_tensor(out=musq, in0=mu_s, in1=mu_s, op=ALU.mult)
                    var_t = bwork.tile([128, 1], FP32, tag="sc", bufs=16, name="var_t")
                    nc.vector.tensor_tensor(out=var_t, in0=b2[:, 321:322], in1=musq,
                                            op=ALU.subtract)
                    lv_t = bwork.tile([128, 1], FP32, tag="sc", bufs=16, name="lv_t")
                    nc.scalar.activation(out=lv_t, in_=var_t, func=AF.Ln,
                                         bias=eps_ln_t)
                    rstd0 = bwork.tile([128, 1], FP32, tag="sc", bufs=16, name="rstd0")
                    nc.scalar.activation(out=rstd0, in_=lv_t, func=AF.Exp, scale=-0.5)
                    # one Newton step: rstd = rstd0*(1.5 - 0.5*(var+eps)*rstd0^2)
                    veps = bwork.tile([128, 1], FP32, tag="sc", bufs=16, name="veps")
                    nc.vector.tensor_scalar(out=veps, in0=var_t, scalar1=EPS_LN,
                                            scalar2=None, op0=ALU.add)
                    rs0 = bwork.tile([128, 1], FP32, tag="sc", bufs=16, name="rs0")
                    nc.vector.tensor_tensor(out=rs0, in0=rstd0, in1=rstd0, op=ALU.mult)
                    nwt = bwork.tile([128, 1], FP32, tag="sc", bufs=16, name="nwt")
                    nc.vector.tensor_tensor(out=nwt, in0=veps, in1=rs0, op=ALU.mult)
                    nwt2 = bwork.tile([128, 1], FP32, tag="sc", bufs=16, name="nwt2")
                    nc.vector.tensor_scalar(out=nwt2, in0=nwt, scalar1=-0.5,
                                            scalar2=1.5, op0=ALU.mult, op1=ALU.add)
                    rstd_t = bwork.tile([128, 1], FP32, tag="sc", bufs=16, name="rstd_t")
                    nc.vector.tensor_tensor(out=rstd_t, in0=rstd0, in1=nwt2, op=ALU.mult)
                    rsq_t = bwork.tile([128, 1], FP32, tag="sc", bufs=16, name="rsq_t")
                    nc.vector.tensor_tensor(out=rsq_t, in0=rstd_t, in1=rstd_t, op=ALU.mult)

                    # |k|^2, |q|^2 (raw) -> exp biases  (-0.5*ss*rsq; 1/sqrt(M) dropped,
                    # cancels between qp and kp in ya = (qp.kptv)/(qp.ks))
                    scr = bwork.tile([128, 320], FP32, tag="scr", name="scr")
                    ss_k = bwork.tile([128, 1], FP32, tag="sc", bufs=16, name="ss_k")
                    nc.scalar.activation(out=scr, in_=b1[:, 160:480], func=AF.Square,
                                         accum_out=ss_k)
                    scr2 = bwork.tile([128, 320], FP32, tag="scr", name="scr2")
                    ss_q = bwork.tile([128, 1], FP32, tag="sc", bufs=16, name="ss_q")
                    nc.scalar.activation(out=scr2, in_=b2[:, 0:320], func=AF.Square,
                                         accum_out=ss_q)
                    bk2p = bwork.tile([128, 1], FP32, tag="sc", bufs=16, name="bk2p")
                    nc.vector.scalar_tensor_tensor(out=bk2p, in0=ss_k, scalar=-0.5,
                                                   in1=rsq_t, op0=ALU.mult, op1=ALU.mult)
                    bk2_t = bwork.tile([128, 1], FP32, tag="sc", bufs=16, name="bk2_t")
                    nc.vector.tensor_scalar(out=bk2_t, in0=bk2p, scalar1=-LNM_HALF,
                                            scalar2=None, op0=ALU.add)
                    bq2p = bwork.tile([128, 1], FP32, tag="sc", bufs=16, name="bq2p")
                    nc.vector.scalar_tensor_tensor(out=bq2p, in0=ss_q, scalar=-0.5,
                                                   in1=rsq_t, op0=ALU.mult, op1=ALU.mult)
                    bq2_t = bwork.tile([128, 1], FP32, tag="sc", bufs=16, name="bq2_t")
                    nc.vector.tensor_scalar(out=bq2_t, in0=bq2p, scalar1=-LNM_HALF,
                                            scalar2=None, op0=ALU.add)

                    # evictions
                    kp_t = bwork.tile([128, 160], BF16, tag="kpt", name="kp_t")
                    nc.scalar.activation(out=kp_t, in_=b0[:, 320:480],
                                         func=AF.Exp, bias=bk2_t, scale=rstd_t)
                    qp_t = bwork.tile([128, 160], BF16, tag="qpt", name="qp_t")
                    nc.scalar.activation(out=qp_t, in_=b1[:, 0:160],
                                         func=AF.Exp, bias=bq2_t, scale=rstd_t)
                    nc.vector.tensor_scalar(out=v_sb[:, i, 0:320], in0=b0[:, 0:320],
                                            scalar1=rstd_t, scalar2=None, op0=ALU.mult)

                    # kptv + ks accumulation (ks via ones column of v_sb)
                    first, last = (i == 0), (i == NTILE - 1)
                    nc.tensor.matmul(psA[:, 0:160], v_sb[:, i, 0:128], kp_t,
                                     start=first, stop=last)
                    nc.tensor.matmul(psA[:, 160:320], v_sb[:, i, 128:256], kp_t,
                                     start=False, stop=last, skip_group_check=True)
                    nc.tensor.matmul(psA[0:65, 320:480], v_sb[:, i, 256:321], kp_t,
                                     start=False, stop=last, skip_group_check=True)

                    # qp transpose to channel-major (PE transpose via identity)
                    pt = tpsum.tile([128, 256], BF16, tag="pt", name="pt")
                    nc.tensor.transpose(pt[:, 0:128], qp_t[:, 0:128], ident_sb)
                    nc.tensor.transpose(pt[0:32, 128:256], qp_t[:, 128:160], ident_sb)
                    nc.vector.tensor_copy(out=qpT0[:, tsl], in_=pt[:, 0:128])
                    nc.vector.tensor_copy(out=qpT1[0:32, tsl], in_=pt[0:32, 128:256])

                # ship partial sums and all-reduce with pair core
                stA = pb.tile([128, 480], FP32, name="stA")
                nc.vector.memset(stA[64:128, 320:480], 0.0)
                nc.vector.tensor_copy(out=stA[:, 0:320], in_=psA[:, 0:320])
                nc.vector.tensor_copy(out=stA[0:65, 320:480], in_=psA[0:65, 320:480])
                nc.sync.dma_start(out=cc_in[:, :], in_=stA)
                nc.gpsimd.collective_compute(
                    "AllReduce", ALU.add,
                    replica_groups=[[0, 1], [2, 3], [4, 5], [6, 7]],
                    ins=[cc_in.opt()], outs=[cc_out.opt()],
                )

            ab_stack.close()   # free tc_sb

            # =================== PHASE C ===================
            with tc.tile_pool(name="pcp", bufs=1) as pcp, \
                 tc.tile_pool(name="cwork", bufs=4) as cwork, \
                 tc.tile_pool(name="cps", bufs=2, space="PSUM") as cps, \
                 tc.tile_pool(name="hps", bufs=1, space="PSUM") as hps, \
                 tc.tile_pool(name="hps2", bufs=3, space="PSUM") as hps2, \
                 tc.tile_pool(name="ops", bufs=2, space="PSUM") as ops, \
                 tc.tile_pool(name="c2w", bufs=3) as c2w:
                # phase-2 weights
                v1g_sb = []
                for i, (o, p) in enumerate(ECH):
                    t = pcp.tile([p, 320], BF16, name=f"v1g_sb_{i}")
                    nc.sync.dma_start(out=t, in_=v1g_ext[o:o + p, :])
                    v1g_sb.append(t)
                w2t_sb = []
                w2t_rows = [(0, 128), (128, 128), (256, 65 if not b2_zero else 64)]
                for i, (o, p) in enumerate(w2t_rows):
                    t = pcp.tile([p, 320], BF16, name=f"w2t_sb_{i}")
                    nc.sync.dma_start(out=t, in_=w2t_ext[o:o + p, :])
                    w2t_sb.append(t)
                pjt_sb = []
                for i, (o, p) in enumerate(ECH):
                    t = pcp.tile([p, 320], BF16, name=f"pjt_sb_{i}")
                    nc.sync.dma_start(out=t, in_=pjt_ext[o:o + p, :])
                    pjt_sb.append(t)

                # collective results: one readback, then slice
                ccf = cwork.tile([128, 480], FP32, tag="ccf", bufs=1, name="ccf")
                nc.sync.dma_start(out=ccf, in_=cc_out[:, :])
                kpe16 = []
                for i, (o, p) in enumerate(ECH):
                    tb = pcp.tile([p, 160], BF16, name=f"kpe16_{i}")
                    nc.vector.tensor_copy(out=tb, in_=ccf[0:p, i * 160:(i + 1) * 160])
                    kpe16.append(tb)
                ksf = cwork.tile([128, 1], FP32, tag="ksf", name="ksf")
                nc.sync.dma_start(out=ksf[0:128, :],
                                  in_=cc_out[64:65, 320:448].rearrange("a b -> b a"))
                ks_col0 = pcp.tile([128, 1], BF16, name="ks_col0")
                nc.vector.tensor_copy(out=ks_col0, in_=ksf)
                ksf1 = cwork.tile([32, 1], FP32, tag="ksf", name="ksf1")
                nc.sync.dma_start(out=ksf1[0:32, :],
                                  in_=cc_out[64:65, 448:480].rearrange("a b -> b a"))
                ks_col1 = pcp.tile([32, 1], BF16, name="ks_col1")
                nc.vector.tensor_copy(out=ks_col1, in_=ksf1)

                # PKV = kptv.T @ proj_w.T  [m, 320o]; col 320 = ks (fuses the
                # D-denominator matmul into the attention matmul)
                pkv_sb = []
                for mi, (mo, mp) in enumerate([(0, 128), (128, 32)]):
                    psPKV = cps.tile([128, 320], FP32, tag="big", name="psPKV")
                    for ec in range(3):
                        nc.tensor.matmul(psPKV[0:mp, :],
                                         kpe16[ec][:, mo:mo + mp], pjt_sb[ec],
                                         start=(ec == 0), stop=(ec == 2))
                    tb = pcp.tile([mp, 321], BF16, name=f"pkv_sb_{mi}")
                    nc.vector.tensor_copy(out=tb[:, 0:320], in_=psPKV[0:mp, :])
                    ksc = (ks_col0 if mi == 0 else ks_col1)
                    nc.vector.tensor_copy(out=tb[:, 320:321], in_=ksc[0:mp, :])
                    pkv_sb.append(tb)

                ya2_sb = pcp.tile([128, NTILE, 320], BF16, name="ya2_sb")
                h2cm0 = pcp.tile([128, T_CORE], BF16, name="h2cm0")
                h2cm12 = pcp.tile([128, 2, T_CORE], BF16, name="h2cm12")

                # ---- C: per 8-tile group (2 blocks): attention tail + LN2 + h2
                #      (C1, nat-log-exp set) then MLP + skip + store (C2, gelu
                #      set) — 2 table loads per group, engines overlap across
                #      the group boundary ----
                GT = 16                                 # tiles per group
                for grp in range(NTILE // GT):
                    s2_8 = cwork.tile([128, GT], FP32, tag="s2_8", bufs=2, name="s2_8")
                    mu2_8 = cwork.tile([128, GT], FP32, tag="mu2_8", bufs=2, name="mu2_8")
                    for t in range(GT):
                        i = grp * GT + t
                        tsl = slice(i * 128, (i + 1) * 128)
                        psP = cps.tile([128, 321], FP32, tag="big", name="psP")
                        nc.tensor.matmul(psP, qpT0[:, tsl], pkv_sb[0],
                                         start=True, stop=False)
                        nc.tensor.matmul(psP, qpT1[0:32, tsl], pkv_sb[1],
                                         start=False, stop=True)

                        dinv = cwork.tile([128, 1], FP32, tag="sc2", bufs=16, name="dinv")
                        nc.vector.tensor_scalar(out=dinv, in0=psP[:, 320:321],
                                                scalar1=EPS_ATTN, scalar2=None,
                                                op0=ALU.add)
                        nc.vector.reciprocal(out=dinv, in_=dinv)

                        if pjb_zero:
                            vadd = v_sb[:, i, 0:320]
                        else:
                            vpj = cwork.tile([128, 320], BF16, tag="vpj", bufs=8, name="vpj")
                            nc.gpsimd.tensor_tensor(out=vpj, in0=v_sb[:, i, 0:320],
                                                    in1=pjb_bc, op=ALU.add)
                            vadd = vpj
                        nc.vector.scalar_tensor_tensor(out=ya2_sb[:, i, :],
                                                       in0=psP[:, 0:320],
                                                       scalar=dinv, in1=vadd,
                                                       op0=ALU.mult, op1=ALU.add,
                                                       accum_out=mu2_8[:, t:t + 1])
                        scr3 = cwork.tile([128, 320], FP32, tag="scr3", bufs=8, name="scr3")
                        nc.scalar.activation(out=scr3, in_=ya2_sb[:, i, :],
                                             func=AF.Square,
                                             accum_out=s2_8[:, t:t + 1])

                    # batched LN2 stats for the group
                    mu8_s = cwork.tile([128, GT], FP32, tag="mu8_s", bufs=2, name="mu8_s")
                    nc.vector.tensor_scalar(out=mu8_s, in0=mu2_8, scalar1=1.0 / EMB,
                                            scalar2=None, op0=ALU.mult)
                    musq8 = cwork.tile([128, GT], FP32, tag="musq8", bufs=2, name="musq8")
                    nc.vector.tensor_tensor(out=musq8, in0=mu8_s, in1=mu8_s, op=ALU.mult)
                    var8 = cwork.tile([128, GT], FP32, tag="var8", bufs=2, name="var8")
                    nc.vector.scalar_tensor_tensor(out=var8, in0=s2_8, scalar=1.0 / EMB,
                                                   in1=musq8, op0=ALU.mult,
                                                   op1=ALU.subtract)
                    lv8 = cwork.tile([128, GT], FP32, tag="lv8", bufs=2, name="lv8")
                    nc.scalar.activation(out=lv8, in_=var8, func=AF.Ln, bias=eps_ln_t)
                    r80 = cwork.tile([128, GT], FP32, tag="r80", bufs=2, name="r80")
                    nc.scalar.activation(out=r80, in_=lv8, func=AF.Exp, scale=-0.5)
                    ve8 = cwork.tile([128, GT], FP32, tag="ve8", bufs=2, name="ve8")
                    nc.vector.tensor_scalar(out=ve8, in0=var8, scalar1=EPS_LN,
                                            scalar2=None, op0=ALU.add)
                    rs8 = cwork.tile([128, GT], FP32, tag="rs8", bufs=2, name="rs8")
                    nc.vector.tensor_tensor(out=rs8, in0=r80, in1=r80, op=ALU.mult)
                    nw8 = cwork.tile([128, GT], FP32, tag="nw8", bufs=2, name="nw8")
                    nc.vector.tensor_tensor(out=nw8, in0=ve8, in1=rs8, op=ALU.mult)
                    nw8b = cwork.tile([128, GT], FP32, tag="nw8b", bufs=2, name="nw8b")
                    nc.vector.tensor_scalar(out=nw8b, in0=nw8, scalar1=-0.5,
                                            scalar2=1.5, op0=ALU.mult, op1=ALU.add)
                    rstd8 = cwork.tile([128, GT], FP32, tag="rstd8", bufs=2, name="rstd8")
                    nc.vector.tensor_tensor(out=rstd8, in0=r80, in1=nw8b, op=ALU.mult)

                    for t in range(GT):
                        i = grp * GT + t
                        tsl = slice(i * 128, (i + 1) * 128)
                        h2_t = cwork.tile([128, 320], BF16, tag="h2t", bufs=8, name="h2_t")
                        nc.gpsimd.tensor_scalar(out=h2_t, in0=ya2_sb[:, i, :],
                                                scalar1=mu8_s[:, t:t + 1],
                                                scalar2=rstd8[:, t:t + 1],
                                                op0=ALU.subtract, op1=ALU.mult)
                        htp = hps.tile([128, 384], BF16, tag="ht", name="htp")
                        nc.tensor.transpose(htp[:, 0:128], h2_t[:, 0:128], ident_sb)
                        nc.tensor.transpose(htp[:, 128:256], h2_t[:, 128:256], ident_sb)
                        nc.tensor.transpose(htp[0:64, 256:384], h2_t[:, 256:320], ident_sb)
                        nc.tensor.matmul(htp[64:128, 256:384], h2_t[:, 256:320], ident_sb,
                                         is_transpose=True, tile_position=(0, 64),
                                         skip_group_check=True)
                        nc.vector.tensor_copy(out=h2cm0[:, tsl], in_=htp[:, 0:128])
                        nc.vector.tensor_copy(out=h2cm12[:, :, tsl],
                                              in_=htp[:, 128:384].rearrange("p (b c) -> p b c", c=128))

                    # ---- C2 for the group's two 512-blocks ----
                    for sub in range(GT // 4):
                        blk = grp * (GT // 4) + sub
                        bsl = slice(blk * 512, (blk + 1) * 512)
                        g_cm = []
                        h2rhs = [h2cm0[:, bsl], h2cm12[:, 0, bsl],
                                 h2cm12[0:64, 1, bsl]]
                        for hc, (ho, hp) in enumerate(HCH):
                            psH = hps2.tile([128, 512], FP32, tag="h", name="psH")
                            for ec, (eo, pe) in enumerate(ECH):
                                nc.tensor.matmul(psH[0:hp, :],
                                                 v1g_sb[ec][:, ho:ho + hp],
                                                 h2rhs[ec],
                                                 start=(ec == 0), stop=(ec == 2))
                            g = c2w.tile([128, 512], BF16, tag=f"g{hc}", name="g")
                            nc.scalar.activation(out=g[0:hp, :], in_=psH[0:hp, :],
                                                 func=AF.Gelu, bias=b1f_sb[0:hp, hc:hc + 1])
                            if hc == 2 and not b2_zero:
                                nc.gpsimd.memset(g[64:65, :], 1.0)
                            g_cm.append(g)
                        for t in range(4):
                            ti = blk * 4 + t
                            tsl2 = slice(t * 128, (t + 1) * 128)
                            osl = slice(ti * 128, (ti + 1) * 128)
                            psO = ops.tile([128, 320], FP32, tag="o", name="psO")
                            for hc, (ho, hp) in enumerate(HCH):
                                gp = 65 if (hc == 2 and not b2_zero) else hp
                                nc.tensor.matmul(psO, g_cm[hc][0:gp, tsl2],
                                                 w2t_sb[hc],
                                                 start=(hc == 0), stop=(hc == 2))
                            outf = c2w.tile([128, 320], FP32, tag="outf", bufs=6, name="outf")
                            nc.vector.tensor_tensor(out=outf, in0=psO,
                                                    in1=ya2_sb[:, ti, :], op=ALU.add)
                            nc.sync.dma_start(out=out_ext[osl, :], in_=outf)

    nc.finalize()
    return nc


# ---------------------------------------------------------------------------
# host entry
# ---------------------------------------------------------------------------

_NC_CACHE = {}


def _get_nc(pjb_zero=True, b2_zero=True):
    key = ("nc", pjb_zero, b2_zero)
    if key not in _NC_CACHE:
        _NC_CACHE[key] = build_nc(pjb_zero, b2_zero)
        _NC_CACHE["nc"] = _NC_CACHE[key]
    return _NC_CACHE[key]


def _numpy_reference(inp):
    """Fallback path (only for nonzero kqv/ln1 bias, never in practice)."""
    from scipy.special import erf as _erf

    x = inp["x"].astype(np.float32)
    Bn, Nn, Cn = x.shape
    Hn = Wn = int(round(math.sqrt(Nn)))
    xi = x.transpose(0, 2, 1).reshape(Bn, Cn, Hn, Wn)

    def conv(xw, w, b, dil, pad):
        xp = np.pad(xw, ((0, 0), (0, 0), (pad, pad), (pad, pad)))
        Ho = Wo = Hn // 2
        cols = np.empty((Bn, Cn * 9, Ho * Wo), np.float32)
        i = 0
        for dy in range(3):
            for dx in range(3):
                sl = xp[:, :, dy * dil:dy * dil + 2 * Ho:2, dx * dil:dx * dil + 2 * Wo:2]
                cols[:, i * Cn:(i + 1) * Cn, :] = sl.reshape(Bn, Cn, -1)
                i += 1
        wm = w.transpose(0, 2, 3, 1).reshape(ED, 9 * Cn)
        return (wm[None] @ cols + b[None, :, None]).reshape(Bn, ED, Ho, Wo)

    def gelu(t):
        return t * 0.5 * (1 + _erf(t / np.sqrt(2.0)))

    y1 = gelu(conv(xi, inp["conv_w1"], inp["conv_b1"], 1, 1))
    y2 = gelu(conv(xi, inp["conv_w2"], inp["conv_b2"], 2, 2))
    y = np.concatenate([y1, y2], 1)
    t = y.reshape(Bn, EMB, -1).transpose(0, 2, 1)

    def ln(z, g, b):
        mu = z.mean(-1, keepdims=True)
        var = z.var(-1)[..., None]
        return (z - mu) / np.sqrt(var + EPS_LN) * g + b

    h = ln(t, inp["ln1_g"], inp["ln1_b"])
    kqv = h @ inp["kqv_w"].T + inp["kqv_b"]
    k, q, v = kqv[..., :EMB], kqv[..., EMB:2 * EMB], kqv[..., 2 * EMB:]
    pwm = inp["perf_w"]

    def prm(z):
        xd = 0.5 * (z * z).sum(-1, keepdims=True)
        return np.exp(z @ pwm.T - xd) / math.sqrt(M)

    kp, qp = prm(k), prm(q)
    D = np.matmul(qp, kp.sum(1)[..., None])
    kptv = np.matmul(v.transpose(0, 2, 1), kp)
    ya = np.matmul(qp, kptv.transpose(0, 2, 1)) / (D + EPS_ATTN)
    ya = v + (ya @ inp["proj_w"].T + inp["proj_b"])
    h2 = ln(ya, inp["ln2_g"], inp["ln2_b"])
    g = gelu(h2 @ inp["mlp_w1"].T + inp["mlp_b1"])
    return (ya + (g @ inp["mlp_w2"].T + inp["mlp_b2"])).astype(np.float32)


def kernel(**inputs):
    inp = {k: np.asarray(v) for k, v in inputs.items()}
    prep = host_prepare_weights(inp)
    if not prep["bias_zero"]:
        return _numpy_reference(inp)

    shared = {
        "wbig0": prep["wbig"][0:128], "wbig1": prep["wbig"][128:256],
        "wbig2": prep["wbig"][256:320],
        "convb": prep["conv_bias"].reshape(320),
        "v1gT": prep["v1gT"], "w2t": prep["w2t"], "projwT": prep["projwT"],
        "b1f": prep["b1f"].reshape(320),
        "ident": np.eye(128, dtype=np.float32).astype(_BF),
    }
    if not prep["pjb_zero"]:
        shared["projb"] = prep["proj_b"].reshape(320)
    conv = prep["conv"]
    for cv in ("c1", "c2"):
        shared[f"{cv}_lo_pairs"] = np.concatenate(
            [w for _, _, w in conv[f"{cv}_lo_pairs"]], axis=1)
        shared[f"{cv}_hi_pairs"] = np.concatenate(
            [w for _, _, w in conv[f"{cv}_hi_pairs"]], axis=1)
    shared["singles_lo"] = np.concatenate(
        [conv["c1_lo_single"][2], conv["c2_lo_single"][2]], axis=0)
    shared["singles_hi"] = np.concatenate(
        [conv["c1_hi_single"][2], conv["c2_hi_single"][2]], axis=0)

    in_maps = []
    for core in range(8):
        stacks = host_prepare_core_input(inp["x"], core)
        m = dict(shared)
        for s, arr in stacks.items():
            m[f"stk_{s}"] = arr
        in_maps.append(m)

    nc = _get_nc(prep["pjb_zero"], prep["b2_zero"])
    res = run_bass_kernel_spmd(nc, in_maps, list(range(8)))
    _NC_CACHE["last_results"] = res
    _NC_CACHE["last_in_maps"] = in_maps
    out = np.empty((B, 16384, EMB), np.float32)
    for core in range(8):
        b, half = core // 2, core % 2
        out[b, half * T_CORE:(half + 1) * T_CORE, :] = res.results[core]["out"]
    return out


# revision 3
# speedup vs baseline: 1.2403x; 1.2403x over previous
"""Trainium2 Bass kernel v2 for nn_BlockRC3 (PRM dilated-conv stem + Token_performer).

Contract: kernel(**inputs) takes FULL unsharded inputs (x [4,65536,64] fp32 + weights),
returns FULL output [4,16384,320] fp32. Data-parallel over 8 NeuronCores, each core
half an image (8192 tokens); one pairwise AllReduce of performer sums (kptv+ks).

v2 changes vs baseline (same math, ~2.5x lower modeled device time):
  - single activation-table set per phase: LN rstd via Ln+Exp (natural_log_exp set)
    instead of Sqrt (+reciprocal), so phase B/C1 never reload ACT tables.
  - v/qp/ya2/h2 stay in SBUF; qp transposed on the PE (128x128 via identity
    matmul) instead of DRAM DMA round trip + DmaTranspose.
  - phase C restructured token-major: per-token scalars (1/D, LN2 stats) are
    per-partition ops, no DMA broadcast round trips; proj bias via vpj,
    mlp2 bias via ones-row in lhsT; output written token-major [8192,320].
  - kptv+ks fused into one PSUM accumulation (ones column in v_sb).
  - elementwise work split across DVE / ACT / Pool(gpsimd, SBUF-only ops).
"""

import math
import os

import numpy as np
import ml_dtypes

import concourse.bacc as bacc
import concourse.mybir as mybir
import concourse.tile as tile
from concourse.bass_utils import run_bass_kernel_spmd

FP32 = mybir.dt.float32
BF16 = mybir.dt.bfloat16
AF = mybir.ActivationFunctionType
ALU = mybir.AluOpType

B, N_IN, CIN = 4, 65536, 64
H = W = 256
EMB, ED, M = 320, 160, 160
T_CORE = 8192            # tokens per core (half image)
NTILE = 64               # 128-token tiles per core
NBLK = 16                # 512-token blocks per core
PR, PC = 66, 130         # parity plane rows/cols (from padded 131x260 input)
PLANE = PR * PC
EPS_LN = 1e-5
EPS_ATTN = 1e-8
LNM_HALF = 0.5 * math.log(M)

WBIG_COLS = 1282
# wbig column layout (3 psum banks):
# b0 <- cols 0:480    : v 0:320 | wtxk 320:480
# b1 <- cols 480:960  : wtxq 0:160 | Zk 160:480
# b2 <- cols 960:1282 : Zq 0:320 | mu 320 | Et2 321 (zero col, t^2 matmul accum)

PERM = np.concatenate([
    np.arange(0, 128),          # conv1 o 0:128
    np.arange(160, 288),        # conv2 o 0:128
    np.arange(128, 160),        # conv1 o 128:160
    np.arange(288, 320),        # conv2 o 128:160
])

_BF = ml_dtypes.bfloat16


def _bf16(a):
    return np.ascontiguousarray(a, dtype=np.float32).astype(_BF)


def _shift_flat(plane, delta):
    out = np.zeros_like(plane)
    out[:, : PLANE - delta] = plane[:, delta:]
    return out


def host_prepare_weights(inp):
    g1 = inp["ln1_g"].astype(np.float64)
    b1 = inp["ln1_b"].astype(np.float64)
    kqv_w = inp["kqv_w"].astype(np.float64)          # [960, 320]
    kqv_b = inp["kqv_b"].astype(np.float64)
    pw = inp["perf_w"].astype(np.float64)            # [160, 320]

    Wp = kqv_w * g1[None, :]
    b_fold = kqv_b + kqv_w @ b1
    s = Wp.sum(axis=1)
    Wpp = Wp - s[:, None] / EMB
    Wk, Wq, Wv = Wpp[0:EMB], Wpp[EMB:2 * EMB], Wpp[2 * EMB:]
    PK = pw @ Wk
    PQ = pw @ Wq
    bias_zero = (np.abs(b_fold).max() == 0.0)

    p = PERM
    Wk_s, Wq_s, Wv_s = Wk[:, p], Wq[:, p], Wv[:, p]
    PK_s, PQ_s = PK[:, p], PQ[:, p]

    wbig = np.zeros((EMB, WBIG_COLS), np.float64)
    wbig[:, 0:320] = Wv_s.T
    wbig[:, 320:480] = PK_s.T
    wbig[:, 480:640] = PQ_s.T
    wbig[:, 640:960] = Wk_s.T
    wbig[:, 960:1280] = Wq_s.T
    wbig[:, 1280] = 1.0   # mu column (device scales by 1/EMB)
    # col 1281 zero: Et2 accumulated by the t^2 matmuls

    # conv weights: tap-pair stationary tiles (same as baseline)
    w1 = inp["conv_w1"].astype(np.float64)
    w2 = inp["conv_w2"].astype(np.float64)

    def tapw(w, dy, dx, osl):
        return np.ascontiguousarray(w[osl, :, dy, dx].T)

    def pairw(w, tapA, tapB, osl):
        return np.concatenate([tapw(w, *tapA, osl), tapw(w, *tapB, osl)], axis=0)

    lo, hi = slice(0, 128), slice(128, 160)
    conv = {}
    c1_pairs = [("S3", 0, (0, 0), (0, 2)), ("S3", 130, (2, 0), (2, 2)),
                ("S4", 1, (0, 1), (2, 1)), ("S5", 130, (1, 0), (1, 2))]
    c1_single = ("S1u", 131, (1, 1))
    c2_pairs = [("S1", 0, (0, 0), (0, 1)), ("S1", 130, (1, 0), (1, 1)),
                ("S1", 260, (2, 0), (2, 1)), ("S2", 2, (0, 2), (1, 2))]
    c2_single = ("S1l", 261, (2, 2))
    for osl, tag in ((lo, "lo"), (hi, "hi")):
        conv[f"c1_{tag}_pairs"] = [(st, off, _bf16(pairw(w1, tA, tB, osl)))
                                   for st, off, tA, tB in c1_pairs]
        conv[f"c1_{tag}_single"] = (c1_single[0], c1_single[1],
                                    _bf16(tapw(w1, *c1_single[2], osl)))
        conv[f"c2_{tag}_pairs"] = [(st, off, _bf16(pairw(w2, tA, tB, osl)))
                                   for st, off, tA, tB in c2_pairs]
        conv[f"c2_{tag}_single"] = (c2_single[0], c2_single[1],
                                    _bf16(tapw(w2, *c2_single[2], osl)))

    cb = np.concatenate([inp["conv_b1"], inp["conv_b2"]]).astype(np.float64)[PERM]

    # LN2 + MLP folds (token-major LN2: only the gain folds into w1)
    g2 = inp["ln2_g"].astype(np.float64)
    b2 = inp["ln2_b"].astype(np.float64)
    w_1 = inp["mlp_w1"].astype(np.float64)           # [320h, 320e]
    b_1 = inp["mlp_b1"].astype(np.float64)
    w_2 = inp["mlp_w2"].astype(np.float64)           # [320o, 320h]
    b_2 = inp["mlp_b2"].astype(np.float64)
    V1g = w_1 * g2[None, :]                          # [320h, 320e]
    b1f = b_1 + w_1 @ b2
    proj_w = inp["proj_w"].astype(np.float64)
    proj_b = inp["proj_b"].astype(np.float64)

    w2t_ext = np.zeros((321, EMB), np.float64)       # [h(+1), o]
    w2t_ext[0:320] = w_2.T
    w2t_ext[320] = b_2

    out = dict(
        wbig=_bf16(wbig),
        conv=conv,
        conv_bias=cb.astype(np.float32),
        bias_zero=bias_zero,
        v1gT=_bf16(V1g.T),                            # [320e, 320h]
        b1f=b1f.astype(np.float32),
        w2t=_bf16(w2t_ext),                           # [321h, 320o]
        projwT=_bf16(proj_w.T),                       # [320e, 320o]
        proj_b=proj_b.astype(np.float32),
        pjb_zero=(np.abs(proj_b).max() == 0.0),
        b2_zero=(np.abs(b_2).max() == 0.0),
    )
    return out


def host_prepare_core_input(x, core):
    b, half = core // 2, core % 2
    xi = np.ascontiguousarray(x[b].reshape(H, W, CIN).transpose(2, 0, 1))
    r0 = 128 * half - 2
    pad = np.zeros((CIN, 131, 260), np.float32)
    rlo, rhi = max(r0, 0), min(r0 + 131, H)
    pad[:, rlo - r0:rhi - r0, 2:258] = xi[:, rlo:rhi, :]
    ee = pad[:, 0::2, 0::2]
    eo = pad[:, 0::2, 1::2]
    oe = np.zeros((CIN, PR, PC), np.float32); oe[:, :65] = pad[:, 1::2, 0::2]
    oo = np.zeros((CIN, PR, PC), np.float32); oo[:, :65] = pad[:, 1::2, 1::2]
    ee = ee.reshape(CIN, PLANE); eo = eo.reshape(CIN, PLANE)
    oe = oe.reshape(CIN, PLANE); oo = oo.reshape(CIN, PLANE)
    stacks = {
        "S1": np.concatenate([ee, _shift_flat(ee, 1)], axis=0),
        "S2": np.concatenate([ee, _shift_flat(ee, 130)], axis=0),
        "S3": np.concatenate([oo, _shift_flat(oo, 1)], axis=0),
        "S4": np.concatenate([oe, _shift_flat(oe, 130)], axis=0),
        "S5": np.concatenate([eo, _shift_flat(eo, 1)], axis=0),
    }
    return {k: _bf16(v) for k, v in stacks.items()}


# ---------------------------------------------------------------------------
# device kernel builder
# ---------------------------------------------------------------------------

def build_nc(pjb_zero=True, b2_zero=True):
    import contextlib
    import concourse.bass as bass

    nc = bacc.Bacc(None, target_bir_lowering=False)

    # Restrict the activation-table chooser to the two sets that each cover a
    # whole phase (indices preserved; other sets emptied so the fixpoint pass
    # can't alternate between per-function tables, which would reload the
    # 1.3us ACT table per tile).
    KEEP = {"natural_log_exp_and_others", "gelu_and_others"}
    from concourse.hw_specs import get_activation_tables
    import bass_rust as _bass_rust_mod

    def _patched_insert_act_table_loads():
        has_activation = any(
            isinstance(i, mybir.InstActivation)
            for b_ in nc.main_func.blocks
            for i in b_.instructions
        )
        if not has_activation:
            return
        tables = [(name, (funcs if name in KEEP else set()))
                  for name, funcs in get_activation_tables(nc.m.arch).items()]
        _bass_rust_mod.insert_act_table_loads(nc, tables)

    nc.insert_act_table_loads = _patched_insert_act_table_loads

    def din(name, shape, dt=BF16):
        return nc.declare_dram_parameter(name, list(shape), dt, isOutput=False)

    stacks_ext = {s: din(f"stk_{s}", [128, PLANE]) for s in ("S1", "S2", "S3", "S4", "S5")}
    wbig_ext = [din("wbig0", [128, WBIG_COLS]), din("wbig1", [128, WBIG_COLS]),
                din("wbig2", [64, WBIG_COLS])]
    cw_ext = {}
    for cv in ("c1", "c2"):
        cw_ext[f"{cv}_lo_pairs"] = din(f"{cv}_lo_pairs", [128, 4 * 128])
        cw_ext[f"{cv}_hi_pairs"] = din(f"{cv}_hi_pairs", [128, 4 * 32])
    cw_ext["singles_lo"] = din("singles_lo", [128, 128])
    cw_ext["singles_hi"] = din("singles_hi", [128, 32])
    convb_ext = din("convb", [320], FP32)
    v1g_ext = din("v1gT", [320, 320])
    w2t_ext = din("w2t", [321, 320])
    pjt_ext = din("projwT", [320, 320])
    b1f_ext = din("b1f", [320], FP32)
    pjb_ext = None if pjb_zero else din("projb", [320], FP32)
    ident_ext = din("ident", [128, 128])

    out_ext = nc.declare_dram_parameter("out", [T_CORE, 320], FP32, isOutput=True)

    ECH = [(0, 128), (128, 128), (256, 64)]
    HCH = [(0, 128), (128, 128), (256, 64)]          # mlp hidden chunks

    with tile.TileContext(nc) as tc:
        with contextlib.ExitStack() as ctx:
            persist = ctx.enter_context(tc.tile_pool(name="persist", bufs=1))
            dram = ctx.enter_context(tc.tile_pool(name="dram", bufs=1, space="DRAM"))

            # ---- persistent constants ----
            eps_ln_t = persist.tile([128, 1], FP32)
            nc.vector.memset(eps_ln_t, EPS_LN)
            ones_t2 = [persist.tile([p, 1], FP32, name=f"ones_t2_{i}")
                       for i, (_, p) in enumerate(ECH)]
            for t in ones_t2:
                nc.vector.memset(t, 1.0 / EMB)
            convb_sb = persist.tile([128, 3], FP32)
            nc.sync.dma_start(out=convb_sb[:, 0:1], in_=convb_ext[0:128].rearrange("(b one) -> b one", one=1))
            nc.sync.dma_start(out=convb_sb[:, 1:2], in_=convb_ext[128:256].rearrange("(b one) -> b one", one=1))
            nc.sync.dma_start(out=convb_sb[0:64, 2:3], in_=convb_ext[256:320].rearrange("(b one) -> b one", one=1))
            b1f_sb = persist.tile([128, 3], FP32)
            for i, (o, p) in enumerate(HCH):
                nc.sync.dma_start(out=b1f_sb[0:p, i:i + 1], in_=b1f_ext[o:o + p].rearrange("(b one) -> b one", one=1))
            ident_sb = persist.tile([128, 128], BF16)
            nc.sync.dma_start(out=ident_sb, in_=ident_ext[:, :])
            if not pjb_zero:
                pjb_bc = persist.tile([128, 320], FP32)
                prow = pjb_ext.rearrange("(one c) -> one c", one=1)[0:1, :]
                nc.sync.dma_start(
                    out=pjb_bc,
                    in_=bass.AP(tensor=prow.tensor, offset=prow.offset,
                                ap=[[0, 128], [1, 320]]))

            cc_in = dram.tile([128, 480], FP32)
            cc_out = dram.tile([128, 480], FP32)

            # ========== v/qpT/ya2/h2 scope (phases B+C) ==========
            pbc = ctx.enter_context(tc.tile_pool(name="pbc", bufs=1))
            v_sb = pbc.tile([128, NTILE, 321], BF16, name="v_sb")
            nc.vector.memset(v_sb[:, :, 320:321], 1.0)   # ks ones column
            qpT0 = pbc.tile([128, T_CORE], BF16, name="qpT0")
            qpT1 = pbc.tile([32, T_CORE], BF16, name="qpT1")

            # ========== tc_sb scope (phases A+B) ==========
            ab_stack = ctx.enter_context(contextlib.ExitStack())
            pab = ab_stack.enter_context(tc.tile_pool(name="pab", bufs=1))
            tc_sb = [pab.tile([p, T_CORE], BF16, name=f"tc_sb_{i}")
                     for i, (_, p) in enumerate(ECH)]

            # =================== PHASE A: conv ===================
            with tc.tile_pool(name="convp", bufs=1) as convp, \
                 tc.tile_pool(name="cpsum", bufs=2, space="PSUM") as cpsum:
                cw = {}
                for k, ext in cw_ext.items():
                    t = convp.tile([128, ext.shape[1]], BF16, name=f"cw_{k}_sb")
                    nc.sync.dma_start(out=t, in_=ext[:, :])
                    cw[k] = t
                stk = {}
                for s in stacks_ext:
                    stk[s] = convp.tile([128, PLANE], BF16, name=f"stk_{s}_sb")
                # chunked + interleaved so conv blk0's rows (every stack's head)
                # land before the tails
                qtr = PLANE // 4
                for q in range(4):
                    lo = q * qtr
                    for s, ext in stacks_ext.items():
                        hi = PLANE if q == 3 else (q + 1) * qtr
                        nc.sync.dma_start(out=stk[s][:, lo:hi], in_=ext[:, lo:hi])

                def stack_view(name):
                    base = stk[name[:2]]
                    r = base.rearrange("p (r c) -> p r c", c=PC)
                    if name.endswith("u"):
                        return r[0:64]
                    if name.endswith("l"):
                        return r[64:128]
                    return r

                def conv_rhs(stname, flat_off, blk):
                    ro, co = divmod(flat_off, PC)
                    v = stack_view(stname)
                    h0 = blk * 4
                    return v[:, h0 + ro:h0 + ro + 4, co:co + 128]

                PAIR_DEFS = {
                    "c1": ([("S3", 0), ("S3", 130), ("S4", 1), ("S5", 130)], ("S1u", 131)),
                    "c2": ([("S1", 0), ("S1", 130), ("S1", 260), ("S2", 2)], ("S1l", 261)),
                }

                for blk in range(NBLK):
                    ps_lo1 = cpsum.tile([128, 512], FP32, tag="pslo1")
                    ps_lo2 = cpsum.tile([128, 512], FP32, tag="pslo2")
                    ps_hi = cpsum.tile([64, 512], FP32, tag="pshi")
                    for cvi, cv in enumerate(("c1", "c2")):
                        pairs, single = PAIR_DEFS[cv]
                        ps = (ps_lo1, ps_lo2)[cvi]
                        wlo = cw[f"{cv}_lo_pairs"]
                        whi = cw[f"{cv}_hi_pairs"]
                        for k, (st, off) in enumerate(pairs):
                            rhs = conv_rhs(st, off, blk)
                            nc.tensor.matmul(ps, wlo[:, k * 128:(k + 1) * 128], rhs,
                                             start=(k == 0), stop=False)
                            nc.tensor.matmul(ps_hi[cvi * 32:(cvi + 1) * 32, :],
                                             whi[:, k * 32:(k + 1) * 32], rhs,
                                             start=(k == 0), stop=False,
                                             tile_position=(0, 32 * cvi))
                        st, off = single
                        rhs = conv_rhs(st, off, blk)
                        wsl = cw["singles_lo"][cvi * 64:(cvi + 1) * 64, :]
                        wsh = cw["singles_hi"][cvi * 64:(cvi + 1) * 64, :]
                        nc.tensor.matmul(ps, wsl, rhs, start=False, stop=True,
                                         tile_position=(64 * cvi, 0))
                        nc.tensor.matmul(ps_hi[cvi * 32:(cvi + 1) * 32, :], wsh, rhs,
                                         start=False, stop=True,
                                         tile_position=(64 * cvi, 32 * cvi))
                    csl = slice(blk * 512, (blk + 1) * 512)
                    nc.scalar.activation(out=tc_sb[0][:, csl], in_=ps_lo1,
                                         func=AF.Gelu, bias=convb_sb[:, 0:1])
                    nc.scalar.activation(out=tc_sb[1][:, csl], in_=ps_lo2,
                                         func=AF.Gelu, bias=convb_sb[:, 1:2])
                    nc.scalar.activation(out=tc_sb[2][:, csl], in_=ps_hi,
                                         func=AF.Gelu, bias=convb_sb[0:64, 2:3])

            # =================== PHASE B: stage-1 + kptv + qp transpose ========
            with tc.tile_pool(name="pb", bufs=1) as pb, \
                 tc.tile_pool(name="spsum", bufs=6, space="PSUM") as spsum, \
                 tc.tile_pool(name="kpsum", bufs=1, space="PSUM") as kpsum, \
                 tc.tile_pool(name="tpsum", bufs=1, space="PSUM") as tpsum, \
                 tc.tile_pool(name="bwork", bufs=4) as bwork:
                wbig_sb = []
                for i, ext in enumerate(wbig_ext):
                    t = pb.tile([ext.shape[0], WBIG_COLS], BF16, name=f"wbig_sb_{i}")
                    nc.sync.dma_start(out=t, in_=ext[:, :])
                    wbig_sb.append(t)

                psA = kpsum.tile([128, 480], FP32)

                for i in range(NTILE):
                    tsl = slice(i * 128, (i + 1) * 128)
                    b0 = spsum.tile([128, 480], FP32, tag="s1", name="b0")
                    b1 = spsum.tile([128, 480], FP32, tag="s1", name="b1")
                    b2 = spsum.tile([128, 480], FP32, tag="s1", name="b2")
                    for kc in range(3):
                        lhsT = tc_sb[kc][:, tsl]
                        nc.tensor.matmul(b0, lhsT, wbig_sb[kc][:, 0:480],
                                         start=(kc == 0), stop=(kc == 2))
                        nc.tensor.matmul(b1, lhsT, wbig_sb[kc][:, 480:960],
                                         start=(kc == 0), stop=(kc == 2))
                        nc.tensor.matmul(b2[:, 0:322], lhsT, wbig_sb[kc][:, 960:1282],
                                         start=(kc == 0), stop=False)
                    for kc in range(3):
                        _, pch = ECH[kc]
                        t2 = bwork.tile([128, 128], FP32, tag="t2", name="t2")
                        nc.gpsimd.tensor_tensor(out=t2[0:pch, :], in0=tc_sb[kc][:, tsl],
                                                in1=tc_sb[kc][:, tsl], op=ALU.mult)
                        nc.tensor.matmul(b2[:, 321:322], t2[0:pch, :], ones_t2[kc],
                                         start=False, stop=(kc == 2))

                    # LN1 stats: var = Et2 - mu^2 ; rstd = exp(-0.5 ln(var+eps))
                    mu_s = bwork.tile([128, 1], FP32, tag="sc", bufs=16, name="mu_s")
                    nc.vector.tensor_scalar(out=mu_s, in0=b2[:, 320:321],
                                            scalar1=1.0 / EMB, scalar2=None, op0=ALU.mult)
                    musq = bwork.tile([128, 1], FP32, tag="sc", bufs=16, name="musq")
                    nc.vector.tensor_tensor(out=musq, in0=mu_s, in1=mu_s, op=ALU.mult)
                    var_t = bwork.tile([128, 1], FP32, tag="sc", bufs=16, name="var_t")
                    nc.vector.tensor_tensor(out=var_t, in0=b2[:, 321:322], in1=musq,
                                            op=ALU.subtract)
                    lv_t = bwork.tile([128, 1], FP32, tag="sc", bufs=16, name="lv_t")
                    nc.scalar.activation(out=lv_t, in_=var_t, func=AF.Ln,
                                         bias=eps_ln_t)
                    rstd0 = bwork.tile([128, 1], FP32, tag="sc", bufs=16, name="rstd0")
                    nc.scalar.activation(out=rstd0, in_=lv_t, func=AF.Exp, scale=-0.5)
                    # one Newton step: rstd = rstd0*(1.5 - 0.5*(var+eps)*rstd0^2)
                    veps = bwork.tile([128, 1], FP32, tag="sc", bufs=16, name="veps")
                    nc.vector.tensor_scalar(out=veps, in0=var_t, scalar1=EPS_LN,
                                            scalar2=None, op0=ALU.add)
                    rs0 = bwork.tile([128, 1], FP32, tag="sc", bufs=16, name="rs0")
                    nc.vector.tensor_tensor(out=rs0, in0=rstd0, in1=rstd0, op=ALU.mult)
                    nwt = bwork.tile([128, 1], FP32, tag="sc", bufs=16, name="nwt")
                    nc.vector.tensor_tensor(out=nwt, in0=veps, in1=rs0, op=ALU.mult)
                    nwt2 = bwork.tile([128, 1], FP32, tag="sc", bufs=16, name="nwt2")
                    nc.vector.tensor_scalar(out=nwt2, in0=nwt, scalar1=-0.5,
                                            scalar2=1.5, op0=ALU.mult, op1=ALU.add)
                    rstd_t = bwork.tile([128, 1], FP32, tag="sc", bufs=16, name="rstd_t")
                    nc.vector.tensor_tensor(out=rstd_t, in0=rstd0, in1=nwt2, op=ALU.mult)
                    rsq_t = bwork.tile([128, 1], FP32, tag="sc", bufs=16, name="rsq_t")
                    nc.vector.tensor_tensor(out=rsq_t, in0=rstd_t, in1=rstd_t, op=ALU.mult)

                    # |k|^2, |q|^2 (raw) -> exp biases  (-0.5*ss*rsq; 1/sqrt(M) dropped,
                    # cancels between qp and kp in ya = (qp.kptv)/(qp.ks))
                    scr = bwork.tile([128, 320], FP32, tag="scr", name="scr")
                    ss_k = bwork.tile([128, 1], FP32, tag="sc", bufs=16, name="ss_k")
                    nc.scalar.activation(out=scr, in_=b1[:, 160:480], func=AF.Square,
                                         accum_out=ss_k)
                    scr2 = bwork.tile([128, 320], FP32, tag="scr", name="scr2")
                    ss_q = bwork.tile([128, 1], FP32, tag="sc", bufs=16, name="ss_q")
                    nc.scalar.activation(out=scr2, in_=b2[:, 0:320], func=AF.Square,
                                         accum_out=ss_q)
                    bk2p = bwork.tile([128, 1], FP32, tag="sc", bufs=16, name="bk2p")
                    nc.vector.scalar_tensor_tensor(out=bk2p, in0=ss_k, scalar=-0.5,
                                                   in1=rsq_t, op0=ALU.mult, op1=ALU.mult)
                    bk2_t = bwork.tile([128, 1], FP32, tag="sc", bufs=16, name="bk2_t")
                    nc.vector.tensor_scalar(out=bk2_t, in0=bk2p, scalar1=-LNM_HALF,
                                            scalar2=None, op0=ALU.add)
                    bq2p = bwork.tile([128, 1], FP32, tag="sc", bufs=16, name="bq2p")
                    nc.vector.scalar_tensor_tensor(out=bq2p, in0=ss_q, scalar=-0.5,
                                                   in1=rsq_t, op0=ALU.mult, op1=ALU.mult)
                    bq2_t = bwork.tile([128, 1], FP32, tag="sc", bufs=16, name="bq2_t")
                    nc.vector.tensor_scalar(out=bq2_t, in0=bq2p, scalar1=-LNM_HALF,
                                            scalar2=None, op0=ALU.add)

                    # evictions
                    kp_t = bwork.tile([128, 160], BF16, tag="kpt", name="kp_t")
                    nc.scalar.activation(out=kp_t, in_=b0[:, 320:480],
                                         func=AF.Exp, bias=bk2_t, scale=rstd_t)
                    qp_t = bwork.tile([128, 160], BF16, tag="qpt", name="qp_t")
                    nc.scalar.activation(out=qp_t, in_=b1[:, 0:160],
                                         func=AF.Exp, bias=bq2_t, scale=rstd_t)
                    nc.vector.tensor_scalar(out=v_sb[:, i, 0:320], in0=b0[:, 0:320],
                                            scalar1=rstd_t, scalar2=None, op0=ALU.mult)

                    # kptv + ks accumulation (ks via ones column of v_sb)
                    first, last = (i == 0), (i == NTILE - 1)
                    nc.tensor.matmul(psA[:, 0:160], v_sb[:, i, 0:128], kp_t,
                                     start=first, stop=last)
                    nc.tensor.matmul(psA[:, 160:320], v_sb[:, i, 128:256], kp_t,
                                     start=False, stop=last, skip_group_check=True)
                    nc.tensor.matmul(psA[0:65, 320:480], v_sb[:, i, 256:321], kp_t,
                                     start=False, stop=last, skip_group_check=True)

                    # qp transpose to channel-major (PE transpose via identity)
                    pt = tpsum.tile([128, 256], BF16, tag="pt", name="pt")
                    nc.tensor.transpose(pt[:, 0:128], qp_t[:, 0:128], ident_sb)
                    nc.tensor.transpose(pt[0:32, 128:256], qp_t[:, 128:160], ident_sb)
                    nc.vector.tensor_copy(out=qpT0[:, tsl], in_=pt[:, 0:128])
                    nc.vector.tensor_copy(out=qpT1[0:32, tsl], in_=pt[0:32, 128:256])

                # ship partial sums and all-reduce with pair core
                stA = pb.tile([128, 480], FP32, name="stA")
                nc.vector.memset(stA[64:128, 320:480], 0.0)
                nc.vector.tensor_copy(out=stA[:, 0:320], in_=psA[:, 0:320])
                nc.vector.tensor_copy(out=stA[0:65, 320:480], in_=psA[0:65, 320:480])
                nc.sync.dma_start(out=cc_in[:, :], in_=stA)
                nc.gpsimd.collective_compute(
                    "AllReduce", ALU.add,
                    replica_groups=[[0, 1], [2, 3], [4, 5], [6, 7]],
                    ins=[cc_in.opt()], outs=[cc_out.opt()],
                )

            ab_stack.close()   # free tc_sb

            # =================== PHASE C ===================
            with tc.tile_pool(name="pcp", bufs=1) as pcp, \
                 tc.tile_pool(name="cwork", bufs=4) as cwork, \
                 tc.tile_pool(name="cps", bufs=3, space="PSUM") as cps, \
                 tc.tile_pool(name="hps", bufs=2, space="PSUM") as hps, \
                 tc.tile_pool(name="hps2", bufs=2, space="PSUM") as hps2, \
                 tc.tile_pool(name="ops", bufs=1, space="PSUM") as ops, \
                 tc.tile_pool(name="c2w", bufs=3) as c2w:
                # phase-2 weights
                v1g_sb = []
                for i, (o, p) in enumerate(ECH):
                    t = pcp.tile([p, 320], BF16, name=f"v1g_sb_{i}")
                    nc.sync.dma_start(out=t, in_=v1g_ext[o:o + p, :])
                    v1g_sb.append(t)
                w2t_sb = []
                w2t_rows = [(0, 128), (128, 128), (256, 65 if not b2_zero else 64)]
                for i, (o, p) in enumerate(w2t_rows):
                    t = pcp.tile([p, 320], BF16, name=f"w2t_sb_{i}")
                    nc.sync.dma_start(out=t, in_=w2t_ext[o:o + p, :])
                    w2t_sb.append(t)
                pjt_sb = []
                for i, (o, p) in enumerate(ECH):
                    t = pcp.tile([p, 320], BF16, name=f"pjt_sb_{i}")
                    nc.sync.dma_start(out=t, in_=pjt_ext[o:o + p, :])
                    pjt_sb.append(t)

                # collective results: one readback, then slice
                ccf = cwork.tile([128, 480], FP32, tag="ccf", bufs=1, name="ccf")
                nc.sync.dma_start(out=ccf, in_=cc_out[:, :])
                kpe16 = []
                for i, (o, p) in enumerate(ECH):
                    tb = pcp.tile([p, 160], BF16, name=f"kpe16_{i}")
                    nc.vector.tensor_copy(out=tb, in_=ccf[0:p, i * 160:(i + 1) * 160])
                    kpe16.append(tb)
                ksf = cwork.tile([128, 1], FP32, tag="ksf", name="ksf")
                nc.sync.dma_start(out=ksf[0:128, :],
                                  in_=cc_out[64:65, 320:448].rearrange("a b -> b a"))
                ks_col0 = pcp.tile([128, 1], BF16, name="ks_col0")
                nc.vector.tensor_copy(out=ks_col0, in_=ksf)
                ksf1 = cwork.tile([32, 1], FP32, tag="ksf", name="ksf1")
                nc.sync.dma_start(out=ksf1[0:32, :],
                                  in_=cc_out[64:65, 448:480].rearrange("a b -> b a"))
                ks_col1 = pcp.tile([32, 1], BF16, name="ks_col1")
                nc.vector.tensor_copy(out=ks_col1, in_=ksf1)

                # PKV = kptv.T @ proj_w.T  [m, 320o]; col 320 = ks (fuses the
                # D-denominator matmul into the attention matmul)
                pkv_sb = []
                for mi, (mo, mp) in enumerate([(0, 128), (128, 32)]):
                    psPKV = cps.tile([128, 320], FP32, tag="big", name="psPKV")
                    for ec in range(3):
                        nc.tensor.matmul(psPKV[0:mp, :],
                                         kpe16[ec][:, mo:mo + mp], pjt_sb[ec],
                                         start=(ec == 0), stop=(ec == 2))
                    tb = pcp.tile([mp, 321], BF16, name=f"pkv_sb_{mi}")
                    nc.vector.tensor_copy(out=tb[:, 0:320], in_=psPKV[0:mp, :])
                    ksc = (ks_col0 if mi == 0 else ks_col1)
                    nc.vector.tensor_copy(out=tb[:, 320:321], in_=ksc[0:mp, :])
                    pkv_sb.append(tb)

                ya2_sb = pcp.tile([128, NTILE, 320], BF16, name="ya2_sb")
                h2cm0 = pcp.tile([128, T_CORE], BF16, name="h2cm0")
                h2cm12 = pcp.tile([128, 2, T_CORE], BF16, name="h2cm12")

                # ---- C: per 8-tile group (2 blocks): attention tail + LN2 + h2
                #      (C1, nat-log-exp set) then MLP + skip + store (C2, gelu
                #      set) — 2 table loads per group, engines overlap across
                #      the group boundary ----
                GT = 16                                 # tiles per group
                for grp in range(NTILE // GT):
                    s2_8 = cwork.tile([128, GT], FP32, tag="s2_8", bufs=2, name="s2_8")
                    mu2_8 = cwork.tile([128, GT], FP32, tag="mu2_8", bufs=2, name="mu2_8")
                    for t in range(GT):
                        i = grp * GT + t
                        tsl = slice(i * 128, (i + 1) * 128)
                        psP = cps.tile([128, 321], FP32, tag="big", name="psP")
                        nc.tensor.matmul(psP, qpT0[:, tsl], pkv_sb[0],
                                         start=True, stop=False)
                        nc.tensor.matmul(psP, qpT1[0:32, tsl], pkv_sb[1],
                                         start=False, stop=True)

                        dinv = cwork.tile([128, 1], FP32, tag="sc2", bufs=24, name="dinv")
                        nc.vector.tensor_scalar(out=dinv, in0=psP[:, 320:321],
                                                scalar1=EPS_ATTN, scalar2=None,
                                                op0=ALU.add)
                        nc.vector.reciprocal(out=dinv, in_=dinv)

                        if pjb_zero:
                            vadd = v_sb[:, i, 0:320]
                        else:
                            vpj = cwork.tile([128, 320], BF16, tag="vpj", bufs=8, name="vpj")
                            nc.gpsimd.tensor_tensor(out=vpj, in0=v_sb[:, i, 0:320],
                                                    in1=pjb_bc, op=ALU.add)
                            vadd = vpj
                        nc.vector.scalar_tensor_tensor(out=ya2_sb[:, i, :],
                                                       in0=psP[:, 0:320],
                                                       scalar=dinv, in1=vadd,
                                                       op0=ALU.mult, op1=ALU.add,
                                                       accum_out=mu2_8[:, t:t + 1])
                        scr3 = cwork.tile([128, 320], FP32, tag="scr3", bufs=4, name="scr3")
                        nc.scalar.activation(out=scr3, in_=ya2_sb[:, i, :],
                                             func=AF.Square,
                                             accum_out=s2_8[:, t:t + 1])

                    # batched LN2 stats for the group
                    mu8_s = cwork.tile([128, GT], FP32, tag="mu8_s", bufs=2, name="mu8_s")
                    nc.vector.tensor_scalar(out=mu8_s, in0=mu2_8, scalar1=1.0 / EMB,
                                            scalar2=None, op0=ALU.mult)
                    musq8 = cwork.tile([128, GT], FP32, tag="musq8", bufs=2, name="musq8")
                    nc.vector.tensor_tensor(out=musq8, in0=mu8_s, in1=mu8_s, op=ALU.mult)
                    var8 = cwork.tile([128, GT], FP32, tag="var8", bufs=2, name="var8")
                    nc.vector.scalar_tensor_tensor(out=var8, in0=s2_8, scalar=1.0 / EMB,
                                                   in1=musq8, op0=ALU.mult,
                                                   op1=ALU.subtract)
                    lv8 = cwork.tile([128, GT], FP32, tag="lv8", bufs=2, name="lv8")
                    nc.scalar.activation(out=lv8, in_=var8, func=AF.Ln, bias=eps_ln_t)
                    r80 = cwork.tile([128, GT], FP32, tag="r80", bufs=2, name="r80")
                    nc.scalar.activation(out=r80, in_=lv8, func=AF.Exp, scale=-0.5)
                    ve8 = cwork.tile([128, GT], FP32, tag="ve8", bufs=2, name="ve8")
                    nc.vector.tensor_scalar(out=ve8, in0=var8, scalar1=EPS_LN,
                                            scalar2=None, op0=ALU.add)
                    rs8 = cwork.tile([128, GT], FP32, tag="rs8", bufs=2, name="rs8")
                    nc.vector.tensor_tensor(out=rs8, in0=r80, in1=r80, op=ALU.mult)
                    nw8 = cwork.tile([128, GT], FP32, tag="nw8", bufs=2, name="nw8")
                    nc.vector.tensor_tensor(out=nw8, in0=ve8, in1=rs8, op=ALU.mult)
                    nw8b = cwork.tile([128, GT], FP32, tag="nw8b", bufs=2, name="nw8b")
                    nc.vector.tensor_scalar(out=nw8b, in0=nw8, scalar1=-0.5,
                                            scalar2=1.5, op0=ALU.mult, op1=ALU.add)
                    rstd8 = cwork.tile([128, GT], FP32, tag="rstd8", bufs=2, name="rstd8")
                    nc.vector.tensor_tensor(out=rstd8, in0=r80, in1=nw8b, op=ALU.mult)

                    for t in range(GT):
                        i = grp * GT + t
                        tsl = slice(i * 128, (i + 1) * 128)
                        h2_t = cwork.tile([128, 320], BF16, tag="h2t", bufs=12, name="h2_t")
                        nc.gpsimd.tensor_scalar(out=h2_t, in0=ya2_sb[:, i, :],
                                                scalar1=mu8_s[:, t:t + 1],
                                                scalar2=rstd8[:, t:t + 1],
                                                op0=ALU.subtract, op1=ALU.mult)
                        htp = hps.tile([128, 384], BF16, tag="ht", name="htp")
                        nc.tensor.transpose(htp[:, 0:128], h2_t[:, 0:128], ident_sb)
                        nc.tensor.transpose(htp[:, 128:256], h2_t[:, 128:256], ident_sb)
                        nc.tensor.transpose(htp[0:64, 256:384], h2_t[:, 256:320], ident_sb)
                        nc.tensor.matmul(htp[64:128, 256:384], h2_t[:, 256:320], ident_sb,
                                         is_transpose=True, tile_position=(0, 64),
                                         skip_group_check=True)
                        nc.vector.tensor_copy(out=h2cm0[:, tsl], in_=htp[:, 0:128])
                        nc.vector.tensor_copy(out=h2cm12[:, :, tsl],
                                              in_=htp[:, 128:384].rearrange("p (b c) -> p b c", c=128))

                    # ---- C2 for the group's two 512-blocks ----
                    for sub in range(GT // 4):
                        blk = grp * (GT // 4) + sub
                        bsl = slice(blk * 512, (blk + 1) * 512)
                        g_cm = []
                        h2rhs = [h2cm0[:, bsl], h2cm12[:, 0, bsl],
                                 h2cm12[0:64, 1, bsl]]
                        for hc, (ho, hp) in enumerate(HCH):
                            psH = hps2.tile([128, 512], FP32, tag="h", name="psH")
                            for ec, (eo, pe) in enumerate(ECH):
                                nc.tensor.matmul(psH[0:hp, :],
                                                 v1g_sb[ec][:, ho:ho + hp],
                                                 h2rhs[ec],
                                                 start=(ec == 0), stop=(ec == 2))
                            g = c2w.tile([128, 512], BF16, tag=f"g{hc}", name="g")
                            nc.scalar.activation(out=g[0:hp, :], in_=psH[0:hp, :],
                                                 func=AF.Gelu, bias=b1f_sb[0:hp, hc:hc + 1])
                            if hc == 2 and not b2_zero:
                                nc.gpsimd.memset(g[64:65, :], 1.0)
                            g_cm.append(g)
                        for t in range(4):
                            ti = blk * 4 + t
                            tsl2 = slice(t * 128, (t + 1) * 128)
                            osl = slice(ti * 128, (ti + 1) * 128)
                            psO = ops.tile([128, 320], FP32, tag="o", name="psO")
                            for hc, (ho, hp) in enumerate(HCH):
                                gp = 65 if (hc == 2 and not b2_zero) else hp
                                nc.tensor.matmul(psO, g_cm[hc][0:gp, tsl2],
                                                 w2t_sb[hc],
                                                 start=(hc == 0), stop=(hc == 2))
                            outf = c2w.tile([128, 320], FP32, tag="outf", bufs=6, name="outf")
                            nc.vector.tensor_tensor(out=outf, in0=psO,
                                                    in1=ya2_sb[:, ti, :], op=ALU.add)
                            nc.sync.dma_start(out=out_ext[osl, :], in_=outf)

    nc.finalize()
    return nc


# ---------------------------------------------------------------------------
# host entry
# ---------------------------------------------------------------------------

_NC_CACHE = {}


def _get_nc(pjb_zero=True, b2_zero=True):
    key = ("nc", pjb_zero, b2_zero)
    if key not in _NC_CACHE:
        _NC_CACHE[key] = build_nc(pjb_zero, b2_zero)
        _NC_CACHE["nc"] = _NC_CACHE[key]
    return _NC_CACHE[key]


def _numpy_reference(inp):
    """Fallback path (only for nonzero kqv/ln1 bias, never in practice)."""
    from scipy.special import erf as _erf

    x = inp["x"].astype(np.float32)
    Bn, Nn, Cn = x.shape
    Hn = Wn = int(round(math.sqrt(Nn)))
    xi = x.transpose(0, 2, 1).reshape(Bn, Cn, Hn, Wn)

    def conv(xw, w, b, dil, pad):
        xp = np.pad(xw, ((0, 0), (0, 0), (pad, pad), (pad, pad)))
        Ho = Wo = Hn // 2
        cols = np.empty((Bn, Cn * 9, Ho * Wo), np.float32)
        i = 0
        for dy in range(3):
            for dx in range(3):
                sl = xp[:, :, dy * dil:dy * dil + 2 * Ho:2, dx * dil:dx * dil + 2 * Wo:2]
                cols[:, i * Cn:(i + 1) * Cn, :] = sl.reshape(Bn, Cn, -1)
                i += 1
        wm = w.transpose(0, 2, 3, 1).reshape(ED, 9 * Cn)
        return (wm[None] @ cols + b[None, :, None]).reshape(Bn, ED, Ho, Wo)

    def gelu(t):
        return t * 0.5 * (1 + _erf(t / np.sqrt(2.0)))

    y1 = gelu(conv(xi, inp["conv_w1"], inp["conv_b1"], 1, 1))
    y2 = gelu(conv(xi, inp["conv_w2"], inp["conv_b2"], 2, 2))
    y = np.concatenate([y1, y2], 1)
    t = y.reshape(Bn, EMB, -1).transpose(0, 2, 1)

    def ln(z, g, b):
        mu = z.mean(-1, keepdims=True)
        var = z.var(-1)[..., None]
        return (z - mu) / np.sqrt(var + EPS_LN) * g + b

    h = ln(t, inp["ln1_g"], inp["ln1_b"])
    kqv = h @ inp["kqv_w"].T + inp["kqv_b"]
    k, q, v = kqv[..., :EMB], kqv[..., EMB:2 * EMB], kqv[..., 2 * EMB:]
    pwm = inp["perf_w"]

    def prm(z):
        xd = 0.5 * (z * z).sum(-1, keepdims=True)
        return np.exp(z @ pwm.T - xd) / math.sqrt(M)

    kp, qp = prm(k), prm(q)
    D = np.matmul(qp, kp.sum(1)[..., None])
    kptv = np.matmul(v.transpose(0, 2, 1), kp)
    ya = np.matmul(qp, kptv.transpose(0, 2, 1)) / (D + EPS_ATTN)
    ya = v + (ya @ inp["proj_w"].T + inp["proj_b"])
    h2 = ln(ya, inp["ln2_g"], inp["ln2_b"])
    g = gelu(h2 @ inp["mlp_w1"].T + inp["mlp_b1"])
    return (ya + (g @ inp["mlp_w2"].T + inp["mlp_b2"])).astype(np.float32)


def kernel(**inputs):
    inp = {k: np.asarray(v) for k, v in inputs.items()}
    prep = host_prepare_weights(inp)
    if not prep["bias_zero"]:
        return _numpy_reference(inp)

    shared = {
        "wbig0": prep["wbig"][0:128], "wbig1": prep["wbig"][128:256],
        "wbig2": prep["wbig"][256:320],
        "convb": prep["conv_bias"].reshape(320),
        "v1gT": prep["v1gT"], "w2t": prep["w2t"], "projwT": prep["projwT"],
        "b1f": prep["b1f"].reshape(320),
        "ident": np.eye(128, dtype=np.float32).astype(_BF),
    }
    if not prep["pjb_zero"]:
        shared["projb"] = prep["proj_b"].reshape(320)
    conv = prep["conv"]
    for cv in ("c1", "c2"):
        shared[f"{cv}_lo_pairs"] = np.concatenate(
            [w for _, _, w in conv[f"{cv}_lo_pairs"]], axis=1)
        shared[f"{cv}_hi_pairs"] = np.concatenate(
            [w for _, _, w in conv[f"{cv}_hi_pairs"]], axis=1)
    shared["singles_lo"] = np.concatenate(
        [conv["c1_lo_single"][2], conv["c2_lo_single"][2]], axis=0)
    shared["singles_hi"] = np.concatenate(
        [conv["c1_hi_single"][2], conv["c2_hi_single"][2]], axis=0)

    in_maps = []
    for core in range(8):
        stacks = host_prepare_core_input(inp["x"], core)
        m = dict(shared)
        for s, arr in stacks.items():
            m[f"stk_{s}"] = arr
        in_maps.append(m)

    nc = _get_nc(prep["pjb_zero"], prep["b2_zero"])
    res = run_bass_kernel_spmd(nc, in_maps, list(range(8)))
    _NC_CACHE["last_results"] = res
    _NC_CACHE["last_in_maps"] = in_maps
    out = np.empty((B, 16384, EMB), np.float32)
    for core in range(8):
        b, half = core // 2, core % 2
        out[b, half * T_CORE:(half + 1) * T_CORE, :] = res.results[core]["out"]
    return out


# revision 4
# speedup vs baseline: 1.4979x; 1.2077x over previous
"""Trainium2 Bass kernel v2 for nn_BlockRC3 (PRM dilated-conv stem + Token_performer).

Contract: kernel(**inputs) takes FULL unsharded inputs (x [4,65536,64] fp32 + weights),
returns FULL output [4,16384,320] fp32. Data-parallel over 8 NeuronCores, each core
half an image (8192 tokens); one pairwise AllReduce of performer sums (kptv+ks).

v2 changes vs baseline (same math, ~2.5x lower modeled device time):
  - single activation-table set per phase: LN rstd via Ln+Exp (natural_log_exp set)
    instead of Sqrt (+reciprocal), so phase B/C1 never reload ACT tables.
  - v/qp/ya2/h2 stay in SBUF; qp transposed on the PE (128x128 via identity
    matmul) instead of DRAM DMA round trip + DmaTranspose.
  - phase C restructured token-major: per-token scalars (1/D, LN2 stats) are
    per-partition ops, no DMA broadcast round trips; proj bias via vpj,
    mlp2 bias via ones-row in lhsT; output written token-major [8192,320].
  - kptv+ks fused into one PSUM accumulation (ones column in v_sb).
  - elementwise work split across DVE / ACT / Pool(gpsimd, SBUF-only ops).
  - D-denominator fused into the attention matmul (ks as PKV column 320);
    LN1/LN2 rsqrt seeded by Ln+Exp tables + one DVE Newton step; kp/qp kept
    at reference scale so the reference's (D + 1e-8) clamping is reproduced
    exactly (it binds for most tokens in this data).
  - collective readback ordered ahead of the phase-C weight DMAs (bulk
    weights on the ACT DMA queue) so the post-AllReduce chain isn't stalled
    behind queued transfers.
"""

import math
import os

import numpy as np
import ml_dtypes

import concourse.bacc as bacc
import concourse.mybir as mybir
import concourse.tile as tile
from concourse.bass_utils import run_bass_kernel_spmd

FP32 = mybir.dt.float32
BF16 = mybir.dt.bfloat16
AF = mybir.ActivationFunctionType
ALU = mybir.AluOpType

B, N_IN, CIN = 4, 65536, 64
H = W = 256
EMB, ED, M = 320, 160, 160
T_CORE = 8192            # tokens per core (half image)
NTILE = 64               # 128-token tiles per core
NBLK = 16                # 512-token blocks per core
PR, PC = 66, 130         # parity plane rows/cols (from padded 131x260 input)
PLANE = PR * PC
EPS_LN = 1e-5
EPS_ATTN = 1e-8
LNM_HALF = 0.5 * math.log(M)

WBIG_COLS = 1282
# wbig column layout (3 psum banks):
# b0 <- cols 0:480    : v 0:320 | wtxk 320:480
# b1 <- cols 480:960  : wtxq 0:160 | Zk 160:480
# b2 <- cols 960:1282 : Zq 0:320 | mu 320 | Et2 321 (zero col, t^2 matmul accum)

PERM = np.concatenate([
    np.arange(0, 128),          # conv1 o 0:128
    np.arange(160, 288),        # conv2 o 0:128
    np.arange(128, 160),        # conv1 o 128:160
    np.arange(288, 320),        # conv2 o 128:160
])

_BF = ml_dtypes.bfloat16


def _bf16(a):
    return np.ascontiguousarray(a, dtype=np.float32).astype(_BF)


def _shift_flat(plane, delta):
    out = np.zeros_like(plane)
    out[:, : PLANE - delta] = plane[:, delta:]
    return out


def host_prepare_weights(inp):
    g1 = inp["ln1_g"].astype(np.float64)
    b1 = inp["ln1_b"].astype(np.float64)
    kqv_w = inp["kqv_w"].astype(np.float64)          # [960, 320]
    kqv_b = inp["kqv_b"].astype(np.float64)
    pw = inp["perf_w"].astype(np.float64)            # [160, 320]

    Wp = kqv_w * g1[None, :]
    b_fold = kqv_b + kqv_w @ b1
    s = Wp.sum(axis=1)
    Wpp = Wp - s[:, None] / EMB
    Wk, Wq, Wv = Wpp[0:EMB], Wpp[EMB:2 * EMB], Wpp[2 * EMB:]
    PK = pw @ Wk
    PQ = pw @ Wq
    bias_zero = (np.abs(b_fold).max() == 0.0)

    p = PERM
    Wk_s, Wq_s, Wv_s = Wk[:, p], Wq[:, p], Wv[:, p]
    PK_s, PQ_s = PK[:, p], PQ[:, p]

    wbig = np.zeros((EMB, WBIG_COLS), np.float64)
    wbig[:, 0:320] = Wv_s.T
    wbig[:, 320:480] = PK_s.T
    wbig[:, 480:640] = PQ_s.T
    wbig[:, 640:960] = Wk_s.T
    wbig[:, 960:1280] = Wq_s.T
    wbig[:, 1280] = 1.0   # mu column (device scales by 1/EMB)
    # col 1281 zero: Et2 accumulated by the t^2 matmuls

    # conv weights: tap-pair stationary tiles (same as baseline)
    w1 = inp["conv_w1"].astype(np.float64)
    w2 = inp["conv_w2"].astype(np.float64)

    def tapw(w, dy, dx, osl):
        return np.ascontiguousarray(w[osl, :, dy, dx].T)

    def pairw(w, tapA, tapB, osl):
        return np.concatenate([tapw(w, *tapA, osl), tapw(w, *tapB, osl)], axis=0)

    lo, hi = slice(0, 128), slice(128, 160)
    conv = {}
    c1_pairs = [("S3", 0, (0, 0), (0, 2)), ("S3", 130, (2, 0), (2, 2)),
                ("S4", 1, (0, 1), (2, 1)), ("S5", 130, (1, 0), (1, 2))]
    c1_single = ("S1u", 131, (1, 1))
    c2_pairs = [("S1", 0, (0, 0), (0, 1)), ("S1", 130, (1, 0), (1, 1)),
                ("S1", 260, (2, 0), (2, 1)), ("S2", 2, (0, 2), (1, 2))]
    c2_single = ("S1l", 261, (2, 2))
    for osl, tag in ((lo, "lo"), (hi, "hi")):
        conv[f"c1_{tag}_pairs"] = [(st, off, _bf16(pairw(w1, tA, tB, osl)))
                                   for st, off, tA, tB in c1_pairs]
        conv[f"c1_{tag}_single"] = (c1_single[0], c1_single[1],
                                    _bf16(tapw(w1, *c1_single[2], osl)))
        conv[f"c2_{tag}_pairs"] = [(st, off, _bf16(pairw(w2, tA, tB, osl)))
                                   for st, off, tA, tB in c2_pairs]
        conv[f"c2_{tag}_single"] = (c2_single[0], c2_single[1],
                                    _bf16(tapw(w2, *c2_single[2], osl)))

    cb = np.concatenate([inp["conv_b1"], inp["conv_b2"]]).astype(np.float64)[PERM]

    # LN2 + MLP folds (token-major LN2: only the gain folds into w1)
    g2 = inp["ln2_g"].astype(np.float64)
    b2 = inp["ln2_b"].astype(np.float64)
    w_1 = inp["mlp_w1"].astype(np.float64)           # [320h, 320e]
    b_1 = inp["mlp_b1"].astype(np.float64)
    w_2 = inp["mlp_w2"].astype(np.float64)           # [320o, 320h]
    b_2 = inp["mlp_b2"].astype(np.float64)
    V1g = w_1 * g2[None, :]                          # [320h, 320e]
    b1f = b_1 + w_1 @ b2
    proj_w = inp["proj_w"].astype(np.float64)
    proj_b = inp["proj_b"].astype(np.float64)

    w2t_ext = np.zeros((321, EMB), np.float64)       # [h(+1), o]
    w2t_ext[0:320] = w_2.T
    w2t_ext[320] = b_2

    out = dict(
        wbig=_bf16(wbig),
        conv=conv,
        conv_bias=cb.astype(np.float32),
        bias_zero=bias_zero,
        v1gT=_bf16(V1g.T),                            # [320e, 320h]
        b1f=b1f.astype(np.float32),
        w2t=_bf16(w2t_ext),                           # [321h, 320o]
        projwT=_bf16(proj_w.T),                       # [320e, 320o]
        proj_b=proj_b.astype(np.float32),
        pjb_zero=(np.abs(proj_b).max() == 0.0),
        b2_zero=(np.abs(b_2).max() == 0.0),
    )
    return out


def host_prepare_core_input(x, core):
    b, half = core // 2, core % 2
    xi = np.ascontiguousarray(x[b].reshape(H, W, CIN).transpose(2, 0, 1))
    r0 = 128 * half - 2
    pad = np.zeros((CIN, 131, 260), np.float32)
    rlo, rhi = max(r0, 0), min(r0 + 131, H)
    pad[:, rlo - r0:rhi - r0, 2:258] = xi[:, rlo:rhi, :]
    ee = pad[:, 0::2, 0::2]
    eo = pad[:, 0::2, 1::2]
    oe = np.zeros((CIN, PR, PC), np.float32); oe[:, :65] = pad[:, 1::2, 0::2]
    oo = np.zeros((CIN, PR, PC), np.float32); oo[:, :65] = pad[:, 1::2, 1::2]
    ee = ee.reshape(CIN, PLANE); eo = eo.reshape(CIN, PLANE)
    oe = oe.reshape(CIN, PLANE); oo = oo.reshape(CIN, PLANE)
    stacks = {
        "S1": np.concatenate([ee, _shift_flat(ee, 1)], axis=0),
        "S2": np.concatenate([ee, _shift_flat(ee, 130)], axis=0),
        "S3": np.concatenate([oo, _shift_flat(oo, 1)], axis=0),
        "S4": np.concatenate([oe, _shift_flat(oe, 130)], axis=0),
        "S5": np.concatenate([eo, _shift_flat(eo, 1)], axis=0),
    }
    return {k: _bf16(v) for k, v in stacks.items()}


# ---------------------------------------------------------------------------
# device kernel builder
# ---------------------------------------------------------------------------

def build_nc(pjb_zero=True, b2_zero=True):
    import contextlib
    import concourse.bass as bass

    nc = bacc.Bacc(None, target_bir_lowering=False)

    # Restrict the activation-table chooser to the two sets that each cover a
    # whole phase (indices preserved; other sets emptied so the fixpoint pass
    # can't alternate between per-function tables, which would reload the
    # 1.3us ACT table per tile).
    KEEP = {"natural_log_exp_and_others", "gelu_and_others"}
    from concourse.hw_specs import get_activation_tables
    import bass_rust as _bass_rust_mod

    def _patched_insert_act_table_loads():
        has_activation = any(
            isinstance(i, mybir.InstActivation)
            for b_ in nc.main_func.blocks
            for i in b_.instructions
        )
        if not has_activation:
            return
        tables = [(name, (funcs if name in KEEP else set()))
                  for name, funcs in get_activation_tables(nc.m.arch).items()]
        _bass_rust_mod.insert_act_table_loads(nc, tables)

    nc.insert_act_table_loads = _patched_insert_act_table_loads

    def din(name, shape, dt=BF16):
        return nc.declare_dram_parameter(name, list(shape), dt, isOutput=False)

    stacks_ext = {s: din(f"stk_{s}", [128, PLANE]) for s in ("S1", "S2", "S3", "S4", "S5")}
    wbig_ext = [din("wbig0", [128, WBIG_COLS]), din("wbig1", [128, WBIG_COLS]),
                din("wbig2", [64, WBIG_COLS])]
    cw_ext = {}
    for cv in ("c1", "c2"):
        cw_ext[f"{cv}_lo_pairs"] = din(f"{cv}_lo_pairs", [128, 4 * 128])
        cw_ext[f"{cv}_hi_pairs"] = din(f"{cv}_hi_pairs", [128, 4 * 32])
    cw_ext["singles_lo"] = din("singles_lo", [128, 128])
    cw_ext["singles_hi"] = din("singles_hi", [128, 32])
    convb_ext = din("convb", [320], FP32)
    v1g_ext = din("v1gT", [320, 320])
    w2t_ext = din("w2t", [321, 320])
    pjt_ext = din("projwT", [320, 320])
    b1f_ext = din("b1f", [320], FP32)
    pjb_ext = None if pjb_zero else din("projb", [320], FP32)
    ident_ext = din("ident", [128, 128])

    out_ext = nc.declare_dram_parameter("out", [T_CORE, 320], FP32, isOutput=True)

    ECH = [(0, 128), (128, 128), (256, 64)]
    HCH = [(0, 128), (128, 128), (256, 64)]          # mlp hidden chunks

    with tile.TileContext(nc) as tc:
        with contextlib.ExitStack() as ctx:
            persist = ctx.enter_context(tc.tile_pool(name="persist", bufs=1))
            dram = ctx.enter_context(tc.tile_pool(name="dram", bufs=1, space="DRAM"))

            # ---- persistent constants ----
            eps_ln_t = persist.tile([128, 1], FP32)
            nc.vector.memset(eps_ln_t, EPS_LN)
            ones_t2 = [persist.tile([p, 1], FP32, name=f"ones_t2_{i}")
                       for i, (_, p) in enumerate(ECH)]
            for t in ones_t2:
                nc.vector.memset(t, 1.0 / EMB)
            convb_sb = persist.tile([128, 3], FP32)
            nc.sync.dma_start(out=convb_sb[:, 0:1], in_=convb_ext[0:128].rearrange("(b one) -> b one", one=1))
            nc.sync.dma_start(out=convb_sb[:, 1:2], in_=convb_ext[128:256].rearrange("(b one) -> b one", one=1))
            nc.sync.dma_start(out=convb_sb[0:64, 2:3], in_=convb_ext[256:320].rearrange("(b one) -> b one", one=1))
            b1f_sb = persist.tile([128, 3], FP32)
            for i, (o, p) in enumerate(HCH):
                nc.sync.dma_start(out=b1f_sb[0:p, i:i + 1], in_=b1f_ext[o:o + p].rearrange("(b one) -> b one", one=1))
            ident_sb = persist.tile([128, 128], BF16)
            nc.sync.dma_start(out=ident_sb, in_=ident_ext[:, :])
            ones_row = persist.tile([1, 1], FP32)
            nc.vector.memset(ones_row, 1.0)
            if not pjb_zero:
                pjb_bc = persist.tile([128, 320], FP32)
                prow = pjb_ext.rearrange("(one c) -> one c", one=1)[0:1, :]
                nc.sync.dma_start(
                    out=pjb_bc,
                    in_=bass.AP(tensor=prow.tensor, offset=prow.offset,
                                ap=[[0, 128], [1, 320]]))

            cc_in = dram.tile([128, 480], FP32)
            cc_out = dram.tile([128, 480], FP32)

            # ========== v/qpT/ya2/h2 scope (phases B+C) ==========
            pbc = ctx.enter_context(tc.tile_pool(name="pbc", bufs=1))
            v_sb = pbc.tile([128, NTILE, 321], BF16, name="v_sb")
            nc.vector.memset(v_sb[:, :, 320:321], 1.0)   # ks ones column
            qpT0 = pbc.tile([128, T_CORE], BF16, name="qpT0")
            qpT1 = pbc.tile([32, T_CORE], BF16, name="qpT1")

            # ========== tc_sb scope (phases A+B) ==========
            ab_stack = ctx.enter_context(contextlib.ExitStack())
            pab = ab_stack.enter_context(tc.tile_pool(name="pab", bufs=1))
            tc_sb = [pab.tile([p, T_CORE], BF16, name=f"tc_sb_{i}")
                     for i, (_, p) in enumerate(ECH)]

            # =================== PHASE A: conv ===================
            with tc.tile_pool(name="convp", bufs=1) as convp, \
                 tc.tile_pool(name="cpsum", bufs=2, space="PSUM") as cpsum:
                cw = {}
                for k, ext in cw_ext.items():
                    t = convp.tile([128, ext.shape[1]], BF16, name=f"cw_{k}_sb")
                    nc.sync.dma_start(out=t, in_=ext[:, :])
                    cw[k] = t
                stk = {}
                for s in stacks_ext:
                    stk[s] = convp.tile([128, PLANE], BF16, name=f"stk_{s}_sb")
                # chunked + interleaved so conv blk0's rows (every stack's head)
                # land before the tails
                qtr = PLANE // 4
                for q in range(4):
                    lo = q * qtr
                    for s, ext in stacks_ext.items():
                        hi = PLANE if q == 3 else (q + 1) * qtr
                        nc.sync.dma_start(out=stk[s][:, lo:hi], in_=ext[:, lo:hi])

                def stack_view(name):
                    base = stk[name[:2]]
                    r = base.rearrange("p (r c) -> p r c", c=PC)
                    if name.endswith("u"):
                        return r[0:64]
                    if name.endswith("l"):
                        return r[64:128]
                    return r

                def conv_rhs(stname, flat_off, blk):
                    ro, co = divmod(flat_off, PC)
                    v = stack_view(stname)
                    h0 = blk * 4
                    return v[:, h0 + ro:h0 + ro + 4, co:co + 128]

                PAIR_DEFS = {
                    "c1": ([("S3", 0), ("S3", 130), ("S4", 1), ("S5", 130)], ("S1u", 131)),
                    "c2": ([("S1", 0), ("S1", 130), ("S1", 260), ("S2", 2)], ("S1l", 261)),
                }

                for blk in range(NBLK):
                    ps_lo1 = cpsum.tile([128, 512], FP32, tag="pslo1")
                    ps_lo2 = cpsum.tile([128, 512], FP32, tag="pslo2")
                    ps_hi = cpsum.tile([64, 512], FP32, tag="pshi")
                    for cvi, cv in enumerate(("c1", "c2")):
                        pairs, single = PAIR_DEFS[cv]
                        ps = (ps_lo1, ps_lo2)[cvi]
                        wlo = cw[f"{cv}_lo_pairs"]
                        whi = cw[f"{cv}_hi_pairs"]
                        for k, (st, off) in enumerate(pairs):
                            rhs = conv_rhs(st, off, blk)
                            nc.tensor.matmul(ps, wlo[:, k * 128:(k + 1) * 128], rhs,
                                             start=(k == 0), stop=False)
                            nc.tensor.matmul(ps_hi[cvi * 32:(cvi + 1) * 32, :],
                                             whi[:, k * 32:(k + 1) * 32], rhs,
                                             start=(k == 0), stop=False,
                                             tile_position=(0, 32 * cvi))
                        st, off = single
                        rhs = conv_rhs(st, off, blk)
                        wsl = cw["singles_lo"][cvi * 64:(cvi + 1) * 64, :]
                        wsh = cw["singles_hi"][cvi * 64:(cvi + 1) * 64, :]
                        nc.tensor.matmul(ps, wsl, rhs, start=False, stop=True,
                                         tile_position=(64 * cvi, 0))
                        nc.tensor.matmul(ps_hi[cvi * 32:(cvi + 1) * 32, :], wsh, rhs,
                                         start=False, stop=True,
                                         tile_position=(64 * cvi, 32 * cvi))
                    csl = slice(blk * 512, (blk + 1) * 512)
                    nc.scalar.activation(out=tc_sb[0][:, csl], in_=ps_lo1,
                                         func=AF.Gelu, bias=convb_sb[:, 0:1])
                    nc.scalar.activation(out=tc_sb[1][:, csl], in_=ps_lo2,
                                         func=AF.Gelu, bias=convb_sb[:, 1:2])
                    nc.scalar.activation(out=tc_sb[2][:, csl], in_=ps_hi,
                                         func=AF.Gelu, bias=convb_sb[0:64, 2:3])

            # =================== PHASE B: stage-1 + kptv + qp transpose ========
            with tc.tile_pool(name="pb", bufs=1) as pb, \
                 tc.tile_pool(name="spsum", bufs=6, space="PSUM") as spsum, \
                 tc.tile_pool(name="kpsum", bufs=1, space="PSUM") as kpsum, \
                 tc.tile_pool(name="tpsum", bufs=1, space="PSUM") as tpsum, \
                 tc.tile_pool(name="bwork", bufs=4) as bwork:
                wbig_sb = []
                for i, ext in enumerate(wbig_ext):
                    t = pb.tile([ext.shape[0], WBIG_COLS], BF16, name=f"wbig_sb_{i}")
                    nc.sync.dma_start(out=t, in_=ext[:, :])
                    wbig_sb.append(t)

                psA = kpsum.tile([128, 480], FP32)

                for i in range(NTILE):
                    tsl = slice(i * 128, (i + 1) * 128)
                    b0 = spsum.tile([128, 480], FP32, tag="s1", name="b0")
                    b1 = spsum.tile([128, 480], FP32, tag="s1", name="b1")
                    b2 = spsum.tile([128, 480], FP32, tag="s1", name="b2")
                    t2s = []
                    for kc in range(3):
                        _, pch = ECH[kc]
                        t2 = bwork.tile([128, 128], FP32, tag="t2", name="t2")
                        nc.gpsimd.tensor_tensor(out=t2[0:pch, :], in0=tc_sb[kc][:, tsl],
                                                in1=tc_sb[kc][:, tsl], op=ALU.mult)
                        t2s.append(t2)
                    # b2 first so the LN1 stats chain overlaps the b1/b0 matmuls
                    for kc in range(3):
                        lhsT = tc_sb[kc][:, tsl]
                        nc.tensor.matmul(b2[:, 0:322], lhsT, wbig_sb[kc][:, 960:1282],
                                         start=(kc == 0), stop=False)
                    for kc in range(3):
                        _, pch = ECH[kc]
                        nc.tensor.matmul(b2[:, 321:322], t2s[kc][0:pch, :], ones_t2[kc],
                                         start=False, stop=(kc == 2))
                    for kc in range(3):
                        lhsT = tc_sb[kc][:, tsl]
                        nc.tensor.matmul(b1, lhsT, wbig_sb[kc][:, 480:960],
                                         start=(kc == 0), stop=(kc == 2))
                    for kc in range(3):
                        lhsT = tc_sb[kc][:, tsl]
                        nc.tensor.matmul(b0, lhsT, wbig_sb[kc][:, 0:480],
                                         start=(kc == 0), stop=(kc == 2))

                    # LN1 stats: var = Et2 - mu^2 ; rstd = exp(-0.5 ln(var+eps))
                    mu_s = bwork.tile([128, 1], FP32, tag="sc", bufs=16, name="mu_s")
                    nc.vector.tensor_scalar(out=mu_s, in0=b2[:, 320:321],
                                            scalar1=1.0 / EMB, scalar2=None, op0=ALU.mult)
                    musq = bwork.tile([128, 1], FP32, tag="sc", bufs=16, name="musq")
                    nc.vector.tensor_tensor(out=musq, in0=mu_s, in1=mu_s, op=ALU.mult)
                    var_t = bwork.tile([128, 1], FP32, tag="sc", bufs=16, name="var_t")
                    nc.vector.tensor_tensor(out=var_t, in0=b2[:, 321:322], in1=musq,
                                            op=ALU.subtract)
                    lv_t = bwork.tile([128, 1], FP32, tag="sc", bufs=16, name="lv_t")
                    nc.scalar.activation(out=lv_t, in_=var_t, func=AF.Ln,
                                         bias=eps_ln_t)
                    rstd0 = bwork.tile([128, 1], FP32, tag="sc", bufs=16, name="rstd0")
                    nc.scalar.activation(out=rstd0, in_=lv_t, func=AF.Exp, scale=-0.5)
                    # one Newton step: rstd = rstd0*(1.5 - 0.5*(var+eps)*rstd0^2)
                    veps = bwork.tile([128, 1], FP32, tag="sc", bufs=16, name="veps")
                    nc.vector.tensor_scalar(out=veps, in0=var_t, scalar1=EPS_LN,
                                            scalar2=None, op0=ALU.add)
                    rs0 = bwork.tile([128, 1], FP32, tag="sc", bufs=16, name="rs0")
                    nc.vector.tensor_tensor(out=rs0, in0=rstd0, in1=rstd0, op=ALU.mult)
                    nwt = bwork.tile([128, 1], FP32, tag="sc", bufs=16, name="nwt")
                    nc.vector.tensor_tensor(out=nwt, in0=veps, in1=rs0, op=ALU.mult)
                    nwt2 = bwork.tile([128, 1], FP32, tag="sc", bufs=16, name="nwt2")
                    nc.vector.tensor_scalar(out=nwt2, in0=nwt, scalar1=-0.5,
                                            scalar2=1.5, op0=ALU.mult, op1=ALU.add)
                    rstd_t = bwork.tile([128, 1], FP32, tag="sc", bufs=16, name="rstd_t")
                    nc.vector.tensor_tensor(out=rstd_t, in0=rstd0, in1=nwt2, op=ALU.mult)
                    rsq_t = bwork.tile([128, 1], FP32, tag="sc", bufs=16, name="rsq_t")
                    nc.vector.tensor_tensor(out=rsq_t, in0=rstd_t, in1=rstd_t, op=ALU.mult)

                    # |k|^2, |q|^2 (raw) -> exp biases  (-0.5*ss*rsq; 1/sqrt(M) dropped,
                    # cancels between qp and kp in ya = (qp.kptv)/(qp.ks))
                    scr = bwork.tile([128, 320], FP32, tag="scr", name="scr")
                    ss_k = bwork.tile([128, 1], FP32, tag="sc", bufs=16, name="ss_k")
                    nc.scalar.activation(out=scr, in_=b1[:, 160:480], func=AF.Square,
                                         accum_out=ss_k)
                    scr2 = bwork.tile([128, 320], FP32, tag="scr", name="scr2")
                    ss_q = bwork.tile([128, 1], FP32, tag="sc", bufs=16, name="ss_q")
                    nc.scalar.activation(out=scr2, in_=b2[:, 0:320], func=AF.Square,
                                         accum_out=ss_q)
                    bk2p = bwork.tile([128, 1], FP32, tag="sc", bufs=16, name="bk2p")
                    nc.vector.scalar_tensor_tensor(out=bk2p, in0=ss_k, scalar=-0.5,
                                                   in1=rsq_t, op0=ALU.mult, op1=ALU.mult)
                    bk2_t = bwork.tile([128, 1], FP32, tag="sc", bufs=16, name="bk2_t")
                    nc.vector.tensor_scalar(out=bk2_t, in0=bk2p, scalar1=-LNM_HALF,
                                            scalar2=None, op0=ALU.add)
                    bq2p = bwork.tile([128, 1], FP32, tag="sc", bufs=16, name="bq2p")
                    nc.vector.scalar_tensor_tensor(out=bq2p, in0=ss_q, scalar=-0.5,
                                                   in1=rsq_t, op0=ALU.mult, op1=ALU.mult)
                    bq2_t = bwork.tile([128, 1], FP32, tag="sc", bufs=16, name="bq2_t")
                    nc.vector.tensor_scalar(out=bq2_t, in0=bq2p, scalar1=-LNM_HALF,
                                            scalar2=None, op0=ALU.add)

                    # evictions
                    kp_t = bwork.tile([128, 160], BF16, tag="kpt", name="kp_t")
                    nc.scalar.activation(out=kp_t, in_=b0[:, 320:480],
                                         func=AF.Exp, bias=bk2_t, scale=rstd_t)
                    qp_t = bwork.tile([128, 160], BF16, tag="qpt", name="qp_t")
                    nc.scalar.activation(out=qp_t, in_=b1[:, 0:160],
                                         func=AF.Exp, bias=bq2_t, scale=rstd_t)
                    nc.vector.tensor_scalar(out=v_sb[:, i, 0:320], in0=b0[:, 0:320],
                                            scalar1=rstd_t, scalar2=None, op0=ALU.mult)

                    # kptv + ks accumulation (ks via ones column of v_sb)
                    first, last = (i == 0), (i == NTILE - 1)
                    nc.tensor.matmul(psA[:, 0:160], v_sb[:, i, 0:128], kp_t,
                                     start=first, stop=last)
                    nc.tensor.matmul(psA[:, 160:320], v_sb[:, i, 128:256], kp_t,
                                     start=False, stop=last, skip_group_check=True)
                    nc.tensor.matmul(psA[0:65, 320:480], v_sb[:, i, 256:321], kp_t,
                                     start=False, stop=last, skip_group_check=True)

                    # qp transpose to channel-major (PE transpose via identity)
                    pt = tpsum.tile([128, 256], BF16, tag="pt", name="pt")
                    nc.tensor.transpose(pt[:, 0:128], qp_t[:, 0:128], ident_sb)
                    nc.tensor.transpose(pt[0:32, 128:256], qp_t[:, 128:160], ident_sb)
                    nc.vector.tensor_copy(out=qpT0[:, tsl], in_=pt[:, 0:128])
                    nc.vector.tensor_copy(out=qpT1[0:32, tsl], in_=pt[0:32, 128:256])

                # ship partial sums and all-reduce with pair core
                stA = pb.tile([128, 480], FP32, name="stA")
                nc.vector.memset(stA[64:128, 320:480], 0.0)
                nc.vector.tensor_copy(out=stA[:, 0:320], in_=psA[:, 0:320])
                nc.vector.tensor_copy(out=stA[0:65, 320:480], in_=psA[0:65, 320:480])
                nc.sync.dma_start(out=cc_in[:, :], in_=stA)
                nc.gpsimd.collective_compute(
                    "AllReduce", ALU.add,
                    replica_groups=[[0, 1], [2, 3], [4, 5], [6, 7]],
                    ins=[cc_in.opt()], outs=[cc_out.opt()],
                )

            ab_stack.close()   # free tc_sb

            # =================== PHASE C ===================
            with tc.tile_pool(name="pcp", bufs=1) as pcp, \
                 tc.tile_pool(name="cwork", bufs=4) as cwork, \
                 tc.tile_pool(name="cps", bufs=3, space="PSUM") as cps, \
                 tc.tile_pool(name="hps", bufs=2, space="PSUM") as hps, \
                 tc.tile_pool(name="hps2", bufs=2, space="PSUM") as hps2, \
                 tc.tile_pool(name="ops", bufs=1, space="PSUM") as ops, \
                 tc.tile_pool(name="c2w", bufs=3) as c2w:
                # collective results: one readback, one bf16 copy; ks rows ->
                # columns via PE transposes (beats two strided DRAM DMAs)
                ccf = cwork.tile([128, 480], FP32, tag="ccf", bufs=1, name="ccf")
                nc.sync.dma_start(out=ccf, in_=cc_out[:, :])
                kpe_all = pcp.tile([128, 480], BF16, name="kpe_all")
                nc.vector.tensor_copy(out=kpe_all, in_=ccf)
                kpe16 = [kpe_all[0:p, i * 160:(i + 1) * 160]
                         for i, (o, p) in enumerate(ECH)]
                ksf = cwork.tile([128, 1], FP32, tag="ksf", name="ksf")
                nc.sync.dma_start(out=ksf[0:128, :],
                                  in_=cc_out[64:65, 320:448].rearrange("a b -> b a"))
                ks_col0 = pcp.tile([128, 1], BF16, name="ks_col0")
                nc.vector.tensor_copy(out=ks_col0, in_=ksf)
                ksf1 = cwork.tile([32, 1], FP32, tag="ksf", name="ksf1")
                nc.sync.dma_start(out=ksf1[0:32, :],
                                  in_=cc_out[64:65, 448:480].rearrange("a b -> b a"))
                ks_col1 = pcp.tile([32, 1], BF16, name="ks_col1")
                nc.vector.tensor_copy(out=ks_col1, in_=ksf1)

                # phase-2 weights (after the collective readback so they don't
                # sit ahead of it in the SP DMA queue; bulk ones on the DVE queue)
                pjt_sb = []
                for i, (o, p) in enumerate(ECH):
                    t = pcp.tile([p, 320], BF16, name=f"pjt_sb_{i}")
                    nc.sync.dma_start(out=t, in_=pjt_ext[o:o + p, :])
                    pjt_sb.append(t)
                v1g_sb = []
                for i, (o, p) in enumerate(ECH):
                    t = pcp.tile([p, 320], BF16, name=f"v1g_sb_{i}")
                    nc.scalar.dma_start(out=t, in_=v1g_ext[o:o + p, :])
                    v1g_sb.append(t)
                w2t_sb = []
                w2t_rows = [(0, 128), (128, 128), (256, 65 if not b2_zero else 64)]
                for i, (o, p) in enumerate(w2t_rows):
                    t = pcp.tile([p, 320], BF16, name=f"w2t_sb_{i}")
                    nc.scalar.dma_start(out=t, in_=w2t_ext[o:o + p, :])
                    w2t_sb.append(t)

                # PKV = kptv.T @ proj_w.T  [m, 320o]; col 320 = ks (fuses the
                # D-denominator matmul into the attention matmul)
                pkv_sb = []
                for mi, (mo, mp) in enumerate([(0, 128), (128, 32)]):
                    psPKV = cps.tile([128, 320], FP32, tag="big", name="psPKV")
                    for ec in range(3):
                        nc.tensor.matmul(psPKV[0:mp, :],
                                         kpe16[ec][:, mo:mo + mp], pjt_sb[ec],
                                         start=(ec == 0), stop=(ec == 2))
                    tb = pcp.tile([mp, 321], BF16, name=f"pkv_sb_{mi}")
                    nc.vector.tensor_copy(out=tb[:, 0:320], in_=psPKV[0:mp, :])
                    ksc = (ks_col0 if mi == 0 else ks_col1)
                    nc.vector.tensor_copy(out=tb[:, 320:321], in_=ksc[0:mp, :])
                    pkv_sb.append(tb)

                ya2_sb = pcp.tile([128, NTILE, 320], BF16, name="ya2_sb")
                h2cm0 = pcp.tile([128, T_CORE], BF16, name="h2cm0")
                h2cm12 = pcp.tile([128, 2, T_CORE], BF16, name="h2cm12")

                # ---- C: per 8-tile group (2 blocks): attention tail + LN2 + h2
                #      (C1, nat-log-exp set) then MLP + skip + store (C2, gelu
                #      set) — 2 table loads per group, engines overlap across
                #      the group boundary ----
                GT = 16                                 # tiles per group
                for grp in range(NTILE // GT):
                    s2_8 = cwork.tile([128, GT], FP32, tag="s2_8", bufs=2, name="s2_8")
                    mu2_8 = cwork.tile([128, GT], FP32, tag="mu2_8", bufs=2, name="mu2_8")
                    for t in range(GT):
                        i = grp * GT + t
                        tsl = slice(i * 128, (i + 1) * 128)
                        psP = cps.tile([128, 321], FP32, tag="big", name="psP")
                        nc.tensor.matmul(psP, qpT0[:, tsl], pkv_sb[0],
                                         start=True, stop=False)
                        nc.tensor.matmul(psP, qpT1[0:32, tsl], pkv_sb[1],
                                         start=False, stop=True)

                        dinv = cwork.tile([128, 1], FP32, tag="sc2", bufs=24, name="dinv")
                        nc.vector.tensor_scalar(out=dinv, in0=psP[:, 320:321],
                                                scalar1=EPS_ATTN, scalar2=None,
                                                op0=ALU.add)
                        nc.vector.reciprocal(out=dinv, in_=dinv)

                        if pjb_zero:
                            vadd = v_sb[:, i, 0:320]
                        else:
                            vpj = cwork.tile([128, 320], BF16, tag="vpj", bufs=8, name="vpj")
                            nc.gpsimd.tensor_tensor(out=vpj, in0=v_sb[:, i, 0:320],
                                                    in1=pjb_bc, op=ALU.add)
                            vadd = vpj
                        nc.vector.scalar_tensor_tensor(out=ya2_sb[:, i, :],
                                                       in0=psP[:, 0:320],
                                                       scalar=dinv, in1=vadd,
                                                       op0=ALU.mult, op1=ALU.add,
                                                       accum_out=mu2_8[:, t:t + 1])
                        scr3 = cwork.tile([128, 320], FP32, tag="scr3", bufs=4, name="scr3")
                        nc.scalar.activation(out=scr3, in_=ya2_sb[:, i, :],
                                             func=AF.Square,
                                             accum_out=s2_8[:, t:t + 1])

                    # batched LN2 stats for the group
                    mu8_s = cwork.tile([128, GT], FP32, tag="mu8_s", bufs=2, name="mu8_s")
                    nc.vector.tensor_scalar(out=mu8_s, in0=mu2_8, scalar1=1.0 / EMB,
                                            scalar2=None, op0=ALU.mult)
                    musq8 = cwork.tile([128, GT], FP32, tag="musq8", bufs=2, name="musq8")
                    nc.vector.tensor_tensor(out=musq8, in0=mu8_s, in1=mu8_s, op=ALU.mult)
                    var8 = cwork.tile([128, GT], FP32, tag="var8", bufs=2, name="var8")
                    nc.vector.scalar_tensor_tensor(out=var8, in0=s2_8, scalar=1.0 / EMB,
                                                   in1=musq8, op0=ALU.mult,
                                                   op1=ALU.subtract)
                    lv8 = cwork.tile([128, GT], FP32, tag="lv8", bufs=2, name="lv8")
                    nc.scalar.activation(out=lv8, in_=var8, func=AF.Ln, bias=eps_ln_t)
                    r80 = cwork.tile([128, GT], FP32, tag="r80", bufs=2, name="r80")
                    nc.scalar.activation(out=r80, in_=lv8, func=AF.Exp, scale=-0.5)
                    ve8 = cwork.tile([128, GT], FP32, tag="ve8", bufs=2, name="ve8")
                    nc.vector.tensor_scalar(out=ve8, in0=var8, scalar1=EPS_LN,
                                            scalar2=None, op0=ALU.add)
                    rs8 = cwork.tile([128, GT], FP32, tag="rs8", bufs=2, name="rs8")
                    nc.vector.tensor_tensor(out=rs8, in0=r80, in1=r80, op=ALU.mult)
                    nw8 = cwork.tile([128, GT], FP32, tag="nw8", bufs=2, name="nw8")
                    nc.vector.tensor_tensor(out=nw8, in0=ve8, in1=rs8, op=ALU.mult)
                    nw8b = cwork.tile([128, GT], FP32, tag="nw8b", bufs=2, name="nw8b")
                    nc.vector.tensor_scalar(out=nw8b, in0=nw8, scalar1=-0.5,
                                            scalar2=1.5, op0=ALU.mult, op1=ALU.add)
                    rstd8 = cwork.tile([128, GT], FP32, tag="rstd8", bufs=2, name="rstd8")
                    nc.vector.tensor_tensor(out=rstd8, in0=r80, in1=nw8b, op=ALU.mult)

                    for t in range(GT):
                        i = grp * GT + t
                        tsl = slice(i * 128, (i + 1) * 128)
                        h2_t = cwork.tile([128, 320], BF16, tag="h2t", bufs=12, name="h2_t")
                        nc.gpsimd.tensor_scalar(out=h2_t, in0=ya2_sb[:, i, :],
                                                scalar1=mu8_s[:, t:t + 1],
                                                scalar2=rstd8[:, t:t + 1],
                                                op0=ALU.subtract, op1=ALU.mult)
                        htp = hps.tile([128, 384], BF16, tag="ht", name="htp")
                        nc.tensor.transpose(htp[:, 0:128], h2_t[:, 0:128], ident_sb)
                        nc.tensor.transpose(htp[:, 128:256], h2_t[:, 128:256], ident_sb)
                        nc.tensor.transpose(htp[0:64, 256:384], h2_t[:, 256:320], ident_sb)
                        nc.tensor.matmul(htp[64:128, 256:384], h2_t[:, 256:320], ident_sb,
                                         is_transpose=True, tile_position=(0, 64),
                                         skip_group_check=True)
                        nc.vector.tensor_copy(out=h2cm0[:, tsl], in_=htp[:, 0:128])
                        nc.vector.tensor_copy(out=h2cm12[:, :, tsl],
                                              in_=htp[:, 128:384].rearrange("p (b c) -> p b c", c=128))

                    # ---- C2 for the group's two 512-blocks ----
                    for sub in range(GT // 4):
                        blk = grp * (GT // 4) + sub
                        bsl = slice(blk * 512, (blk + 1) * 512)
                        g_cm = []
                        h2rhs = [h2cm0[:, bsl], h2cm12[:, 0, bsl],
                                 h2cm12[0:64, 1, bsl]]
                        for hc, (ho, hp) in enumerate(HCH):
                            psH = hps2.tile([128, 512], FP32, tag="h", name="psH")
                            for ec, (eo, pe) in enumerate(ECH):
                                nc.tensor.matmul(psH[0:hp, :],
                                                 v1g_sb[ec][:, ho:ho + hp],
                                                 h2rhs[ec],
                                                 start=(ec == 0), stop=(ec == 2))
                            g = c2w.tile([128, 512], BF16, tag=f"g{hc}", name="g")
                            nc.scalar.activation(out=g[0:hp, :], in_=psH[0:hp, :],
                                                 func=AF.Gelu, bias=b1f_sb[0:hp, hc:hc + 1])
                            if hc == 2 and not b2_zero:
                                nc.gpsimd.memset(g[64:65, :], 1.0)
                            g_cm.append(g)
                        for t in range(4):
                            ti = blk * 4 + t
                            tsl2 = slice(t * 128, (t + 1) * 128)
                            osl = slice(ti * 128, (ti + 1) * 128)
                            psO = ops.tile([128, 320], FP32, tag="o", name="psO")
                            for hc, (ho, hp) in enumerate(HCH):
                                gp = 65 if (hc == 2 and not b2_zero) else hp
                                nc.tensor.matmul(psO, g_cm[hc][0:gp, tsl2],
                                                 w2t_sb[hc],
                                                 start=(hc == 0), stop=(hc == 2))
                            outf = c2w.tile([128, 320], FP32, tag="outf", bufs=6, name="outf")
                            nc.vector.tensor_tensor(out=outf, in0=psO,
                                                    in1=ya2_sb[:, ti, :], op=ALU.add)
                            nc.sync.dma_start(out=out_ext[osl, :], in_=outf)

    nc.finalize()
    return nc


# ---------------------------------------------------------------------------
# host entry
# ---------------------------------------------------------------------------

_NC_CACHE = {}


def _get_nc(pjb_zero=True, b2_zero=True):
    key = ("nc", pjb_zero, b2_zero)
    if key not in _NC_CACHE:
        _NC_CACHE[key] = build_nc(pjb_zero, b2_zero)
        _NC_CACHE["nc"] = _NC_CACHE[key]
    return _NC_CACHE[key]


def _numpy_reference(inp):
    """Fallback path (only for nonzero kqv/ln1 bias, never in practice)."""
    from scipy.special import erf as _erf

    x = inp["x"].astype(np.float32)
    Bn, Nn, Cn = x.shape
    Hn = Wn = int(round(math.sqrt(Nn)))
    xi = x.transpose(0, 2, 1).reshape(Bn, Cn, Hn, Wn)

    def conv(xw, w, b, dil, pad):
        xp = np.pad(xw, ((0, 0), (0, 0), (pad, pad), (pad, pad)))
        Ho = Wo = Hn // 2
        cols = np.empty((Bn, Cn * 9, Ho * Wo), np.float32)
        i = 0
        for dy in range(3):
            for dx in range(3):
                sl = xp[:, :, dy * dil:dy * dil + 2 * Ho:2, dx * dil:dx * dil + 2 * Wo:2]
                cols[:, i * Cn:(i + 1) * Cn, :] = sl.reshape(Bn, Cn, -1)
                i += 1
        wm = w.transpose(0, 2, 3, 1).reshape(ED, 9 * Cn)
        return (wm[None] @ cols + b[None, :, None]).reshape(Bn, ED, Ho, Wo)

    def gelu(t):
        return t * 0.5 * (1 + _erf(t / np.sqrt(2.0)))

    y1 = gelu(conv(xi, inp["conv_w1"], inp["conv_b1"], 1, 1))
    y2 = gelu(conv(xi, inp["conv_w2"], inp["conv_b2"], 2, 2))
    y = np.concatenate([y1, y2], 1)
    t = y.reshape(Bn, EMB, -1).transpose(0, 2, 1)

    def ln(z, g, b):
        mu = z.mean(-1, keepdims=True)
        var = z.var(-1)[..., None]
        return (z - mu) / np.sqrt(var + EPS_LN) * g + b

    h = ln(t, inp["ln1_g"], inp["ln1_b"])
    kqv = h @ inp["kqv_w"].T + inp["kqv_b"]
    k, q, v = kqv[..., :EMB], kqv[..., EMB:2 * EMB], kqv[..., 2 * EMB:]
    pwm = inp["perf_w"]

    def prm(z):
        xd = 0.5 * (z * z).sum(-1, keepdims=True)
        return np.exp(z @ pwm.T - xd) / math.sqrt(M)

    kp, qp = prm(k), prm(q)
    D = np.matmul(qp, kp.sum(1)[..., None])
    kptv = np.matmul(v.transpose(0, 2, 1), kp)
    ya = np.matmul(qp, kptv.transpose(0, 2, 1)) / (D + EPS_ATTN)
    ya = v + (ya @ inp["proj_w"].T + inp["proj_b"])
    h2 = ln(ya, inp["ln2_g"], inp["ln2_b"])
    g = gelu(h2 @ inp["mlp_w1"].T + inp["mlp_b1"])
    return (ya + (g @ inp["mlp_w2"].T + inp["mlp_b2"])).astype(np.float32)


def kernel(**inputs):
    inp = {k: np.asarray(v) for k, v in inputs.items()}
    prep = host_prepare_weights(inp)
    if not prep["bias_zero"]:
        return _numpy_reference(inp)

    shared = {
        "wbig0": prep["wbig"][0:128], "wbig1": prep["wbig"][128:256],
        "wbig2": prep["wbig"][256:320],
        "convb": prep["conv_bias"].reshape(320),
        "v1gT": prep["v1gT"], "w2t": prep["w2t"], "projwT": prep["projwT"],
        "b1f": prep["b1f"].reshape(320),
        "ident": np.eye(128, dtype=np.float32).astype(_BF),
    }
    if not prep["pjb_zero"]:
        shared["projb"] = prep["proj_b"].reshape(320)
    conv = prep["conv"]
    for cv in ("c1", "c2"):
        shared[f"{cv}_lo_pairs"] = np.concatenate(
            [w for _, _, w in conv[f"{cv}_lo_pairs"]], axis=1)
        shared[f"{cv}_hi_pairs"] = np.concatenate(
            [w for _, _, w in conv[f"{cv}_hi_pairs"]], axis=1)
    shared["singles_lo"] = np.concatenate(
        [conv["c1_lo_single"][2], conv["c2_lo_single"][2]], axis=0)
    shared["singles_hi"] = np.concatenate(
        [conv["c1_hi_single"][2], conv["c2_hi_single"][2]], axis=0)

    in_maps = []
    for core in range(8):
        stacks = host_prepare_core_input(inp["x"], core)
        m = dict(shared)
        for s, arr in stacks.items():
            m[f"stk_{s}"] = arr
        in_maps.append(m)

    nc = _get_nc(prep["pjb_zero"], prep["b2_zero"])
    res = run_bass_kernel_spmd(nc, in_maps, list(range(8)))
    _NC_CACHE["last_results"] = res
    _NC_CACHE["last_in_maps"] = in_maps
    out = np.empty((B, 16384, EMB), np.float32)
    for core in range(8):
        b, half = core // 2, core % 2
        out[b, half * T_CORE:(half + 1) * T_CORE, :] = res.results[core]["out"]
    return out
